# revision 16
# baseline (speedup 1.0000x reference)
"""Trainium2 Bass kernel for nn_Block (BatchNorm -> QKV -> causal MHA + gelu gate -> out proj + residual).

Contract: kernel(**inputs) takes FULL unsharded inputs (np arrays, keys as in
setup_inputs()) and returns the FULL output (4, 2048, 384) float32.

Sharding: 8 cores; core c handles batch b=c//2 and head-half s=c%2 (4 of 8 heads).
Both cores of a batch pair compute the full residual output for their batch
(symmetric SPMD program); o-halves are exchanged with a 2-rank AllGather.
BatchNorm statistics are computed replicated on every core from a
column-permuted transposed copy of the full x (own batch's 2048 columns first,
so the program is identical across cores).
"""

import math
import os

import numpy as np
import ml_dtypes

import bass_rust as _bass_rust
import concourse.bass as bass
import concourse.tile as tile
import concourse.mybir as mybir
from concourse.bass_utils import run_bass_kernel_spmd

BF16 = mybir.dt.bfloat16
F32 = mybir.dt.float32
AF = mybir.ActivationFunctionType
ALU = mybir.AluOpType

# Problem constants (hardcoded per harness contract)
B, L, D = 4, 2048, 384
HEADS = 8
HD = 48  # head dim
H_LOC = 4  # heads per core
EXPAND = 2
EPS = 1e-5
NTOK = B * L  # tokens for batchnorm stats
N_CORES = 8
QT = 512  # q tile width
NQT = L // QT  # 4 q tiles
NKT = L // 128  # 16 k tiles
DC = D // 128  # 3 d-chunks
SCALE = 1.0 / math.sqrt(HD)

# qkvwT packed column layout: [Qpack 256 | Kpack 256 | V 192 | pre 384]
QCOL, KCOL, VCOL, PCOL = 0, 256, 512, 704
QKVW_COLS = 1088

LAST_EXEC_TIME_NS = None
LAST_RESULTS = None

_PROGRAM_CACHE = {}


def _build_program(split_waits=True, gelu_func=None):
    gelu = gelu_func or AF.Gelu_apprx_tanh
    nc = bass.Bass()

    # ---- I/O ----
    xT = nc.dram_tensor("xT", [D, NTOK], BF16, kind="ExternalInput")
    xres = nc.dram_tensor("xres", [L, D], F32, kind="ExternalInput")
    gamma = nc.dram_tensor("gamma", [128, DC], F32, kind="ExternalInput")
    beta = nc.dram_tensor("beta", [128, DC], F32, kind="ExternalInput")
    qkvwT = nc.dram_tensor("qkvwT", [D, QKVW_COLS], BF16, kind="ExternalInput")
    qkvb = nc.dram_tensor("qkvb", [128, 4], F32, kind="ExternalInput")
    preb = nc.dram_tensor("preb", [128, DC], F32, kind="ExternalInput")
    vbias = nc.dram_tensor("vbias", [128, 2], F32, kind="ExternalInput")
    outwT = nc.dram_tensor("outwT", [7, 128, D], BF16, kind="ExternalInput")
    outb = nc.dram_tensor("outb", [1, D], BF16, kind="ExternalInput")
    masks = nc.dram_tensor("masks", [2, 128, 1024], BF16, kind="ExternalInput")
    ones_row = nc.dram_tensor("ones_row", [1, L], BF16, kind="ExternalInput")
    out = nc.dram_tensor("out", [L, D], F32, kind="ExternalOutput")

    with tile.TileContext(nc) as tc:
        with (
            tc.tile_pool(name="const", bufs=1) as const,
            tc.tile_pool(name="work", bufs=1) as work,
            tc.tile_pool(name="ptp", bufs=3) as ptp,
            tc.tile_pool(name="scr", bufs=2) as scr,
            tc.tile_pool(name="outp", bufs=3) as outpool,
            tc.tile_pool(name="nrm", bufs=1) as nrm,
            tc.tile_pool(name="psA", bufs=2, space="PSUM") as psA,
            tc.tile_pool(name="psB", bufs=2, space="PSUM") as psB,
            tc.tile_pool(name="psO", bufs=2, space="PSUM") as psO,
            tc.tile_pool(name="dram", bufs=1, space="DRAM") as dram,
        ):
            # ---- load inputs to SBUF ----
            xT_sb = []
            for c in range(DC):
                t = const.tile([128, NTOK], BF16, name=f"xT_sb{c}", tag=f"xT_sb{c}")
                for i in range(4):
                    w = NTOK // 4
                    nc.sync.dma_start(
                        out=t[:, w * i : w * (i + 1)],
                        in_=xT[128 * c : 128 * (c + 1), w * i : w * (i + 1)],
                    )
                xT_sb.append(t)
            xres_sb = []
            for t16 in range(16):
                t = const.tile([128, D], F32, name=f"xres{t16}", tag=f"xres{t16}")
                nc.sync.dma_start(out=t, in_=xres[128 * t16 : 128 * (t16 + 1), :])
                xres_sb.append(t)
            w_sb = []
            for c in range(DC):
                t = const.tile([128, QKVW_COLS], BF16, name=f"w{c}", tag=f"w{c}")
                nc.sync.dma_start(out=t, in_=qkvwT[128 * c : 128 * (c + 1), :])
                w_sb.append(t)
            outw_sb = []
            for i in range(7):
                t = const.tile([128, D], BF16, name=f"outw{i}", tag=f"outw{i}")
                nc.sync.dma_start(out=t, in_=outwT[i])
                outw_sb.append(t)
            outb_sb = const.tile([1, D], BF16, name="outb_sb", tag="outb_sb")
            nc.sync.dma_start(out=outb_sb, in_=outb[:, :])
            masks_sb = []
            for i in range(2):
                t = const.tile([128, 1024], BF16, name=f"mask{i}", tag=f"mask{i}")
                nc.sync.dma_start(out=t, in_=masks[i])
                masks_sb.append(t)
            ones_sb = const.tile([1, L], BF16, name="ones_sb", tag="ones_sb")
            nc.sync.dma_start(out=ones_sb, in_=ones_row[:, :])
            gamma_sb = const.tile([128, DC], F32, name="gamma_sb", tag="gamma_sb")
            nc.sync.dma_start(out=gamma_sb, in_=gamma[:, :])
            beta_sb = const.tile([128, DC], F32, name="beta_sb", tag="beta_sb")
            nc.sync.dma_start(out=beta_sb, in_=beta[:, :])
            qkvb_sb = const.tile([128, 4], F32, name="qkvb_sb", tag="qkvb_sb")
            nc.sync.dma_start(out=qkvb_sb, in_=qkvb[:, :])
            preb_sb = const.tile([128, DC], F32, name="preb_sb", tag="preb_sb")
            nc.sync.dma_start(out=preb_sb, in_=preb[:, :])
            vbias_sb = const.tile([128, 2], F32, name="vbias_sb", tag="vbias_sb")
            nc.sync.dma_start(out=vbias_sb, in_=vbias[:, :])

            # ---- batchnorm statistics (replicated over full x) ----
            # bn_stats per 512-col chunk, bn_aggr combines -> (mean, var) per channel
            mv = work.tile([128, DC, 2], F32, name="mv", tag="mv")
            NCH = NTOK // 512
            for c in range(DC):
                st6 = work.tile([128, NCH, 6], F32, name=f"st6_{c}", tag=f"st6_{c}")
                for i in range(NCH):
                    nc.vector.bn_stats(
                        out=st6[:, i, :], in_=xT_sb[c][:, 512 * i : 512 * (i + 1)]
                    )
                nc.vector.bn_aggr(out=mv[:, c, :], in_=st6)
            mean = mv[:, :, 0]
            var = mv[:, :, 1]
            # scale s = rstd*gamma (rstd = exp(-0.5*ln(var+eps))), bias bb = beta - mean*s
            eps_sb = work.tile([128, 1], F32, name="eps_sb", tag="eps_sb")
            nc.vector.memset(eps_sb, EPS)
            lnv = work.tile([128, DC], F32, name="lnv", tag="lnv")
            nc.scalar.activation(out=lnv, in_=var, func=AF.Ln, bias=eps_sb, scale=1.0)
            rstd = work.tile([128, DC], F32, name="rstd", tag="rstd")
            nc.scalar.activation(out=rstd, in_=lnv, func=AF.Exp, scale=-0.5)
            s_sb = work.tile([128, DC], F32, name="s_sb", tag="s_sb")
            nc.vector.tensor_mul(out=s_sb, in0=rstd, in1=gamma_sb)
            bb_sb = work.tile([128, DC], F32, name="bb_sb", tag="bb_sb")
            nc.vector.scalar_tensor_tensor(
                out=bb_sb, in0=mean, scalar=-1.0, in1=s_sb, op0=ALU.mult, op1=ALU.mult,
            )  # -mean*s
            nc.vector.tensor_add(out=bb_sb, in0=bb_sb, in1=beta_sb)

            # ---- normalize own batch -> hT (bf16) ----
            hT_sb = []
            for c in range(DC):
                t = work.tile([128, L], BF16, name=f"hT{c}", tag=f"hT{c}")
                nc.vector.tensor_scalar(
                    out=t, in0=xT_sb[c][:, 0:L], scalar1=s_sb[:, c : c + 1],
                    scalar2=bb_sb[:, c : c + 1], op0=ALU.mult, op1=ALU.add,
                )
                hT_sb.append(t)

            # ---- QKV projections ----
            # QT/KT: transposed [outch, tok]; heads packed 2-per-tile (48+48+32pad)
            QT_sb = [work.tile([128, L], BF16, name=f"QTt{m}", tag=f"QTt{m}") for m in range(2)]
            KT_sb = [work.tile([128, L], BF16, name=f"KTt{m}", tag=f"KTt{m}") for m in range(2)]
            for dst, col0, bcol0 in ((QT_sb, QCOL, 0), (KT_sb, KCOL, 2)):
                for m in range(2):
                    for tt in range(NQT):
                        ps = psB.tile([128, QT], F32, name="ps_qk", tag="psB")
                        for c in range(DC):
                            nc.tensor.matmul(
                                out=ps,
                                lhsT=w_sb[c][:, col0 + 128 * m : col0 + 128 * (m + 1)],
                                rhs=hT_sb[c][:, QT * tt : QT * (tt + 1)],
                                start=(c == 0), stop=(c == DC - 1),
                            )
                        nc.vector.tensor_scalar(
                            out=dst[m][:, QT * tt : QT * (tt + 1)], in0=ps,
                            scalar1=qkvb_sb[:, bcol0 + m : bcol0 + m + 1], scalar2=None,
                            op0=ALU.add,
                        )
            # V natural [tok, 4 heads * 49] with ones column per head
            V_sb = []
            for t16 in range(16):
                vt = work.tile([128, H_LOC * 65], BF16, name=f"V{t16}", tag=f"V{t16}")
                nc.vector.memset(vt, 1.0)
                V_sb.append(vt)
            for t16 in range(16):
                ps = psB.tile([128, H_LOC * HD], F32, name="ps_v", tag="psB")
                for c in range(DC):
                    nc.tensor.matmul(
                        out=ps,
                        lhsT=hT_sb[c][:, 128 * t16 : 128 * (t16 + 1)],
                        rhs=w_sb[c][:, VCOL : VCOL + H_LOC * HD],
                        start=(c == 0), stop=(c == DC - 1),
                    )
                dstv = V_sb[t16].rearrange("p (h d) -> p h d", h=H_LOC)[:, :, 0:HD]
                nc.vector.tensor_copy(
                    out=dstv, in_=ps.rearrange("p (h d) -> p h d", h=H_LOC)
                )

            # ---- attention (4 local heads, causal) ----
            OT_sb = [work.tile([128, L], BF16, name=f"OTt{m}", tag=f"OTt{m}") for m in range(2)]
            for m in range(2):
                nc.vector.memset(OT_sb[m], 0.0)
            for m in range(2):
                denom = nrm.tile([64, L], F32, name="denom", tag="denom")
                nc.vector.memset(denom, 0.0)
                for hh in range(2):
                    h, ko = 2 * m + hh, 64 * hh
                    for j in range(NQT):
                        ext = 4 * (j + 1)  # k tiles for this q tile
                        ot = psO.tile([65, QT], F32, name="ps_ot", tag="psO")
                        for p in range(ext // 2):
                            st = psA.tile([128, 1024], F32, name="ps_st", tag="psA")
                            for half in range(2):
                                t = 2 * p + half
                                nc.tensor.matmul(
                                    out=st[:, QT * half : QT * (half + 1)],
                                    lhsT=KT_sb[m][ko : ko + HD, 128 * t : 128 * (t + 1)],
                                    rhs=QT_sb[m][ko : ko + HD, QT * j : QT * (j + 1)],
                                    start=True, stop=True,
                                )
                            pt = ptp.tile([128, 1024], BF16, name="pt", tag="pt")
                            nc.scalar.activation(out=pt, in_=st, func=AF.Exp, scale=SCALE)
                            if p == 2 * j:
                                nc.vector.tensor_mul(out=pt, in0=pt, in1=masks_sb[0])
                            elif p == 2 * j + 1:
                                nc.vector.tensor_mul(out=pt, in0=pt, in1=masks_sb[1])
                            for half in range(2):
                                t = 2 * p + half
                                nc.tensor.matmul(
                                    out=ot,
                                    lhsT=V_sb[t][:, 65 * h : 65 * (h + 1)],
                                    rhs=pt[:, QT * half : QT * (half + 1)],
                                    start=(p == 0 and half == 0),
                                    stop=(p == ext // 2 - 1 and half == 1),
                                )
                        # reciprocal of softmax denominator (row 48), unnormalized o out
                        nc.vector.reciprocal(
                            out=denom[32 * hh : 32 * hh + 1, QT * j : QT * (j + 1)],
                            in_=ot[64:65, :],
                        )
                        nc.vector.tensor_copy(
                            out=OT_sb[m][ko : ko + HD, QT * j : QT * (j + 1)],
                            in_=ot[0:HD, :],
                        )
                # broadcast 1/denom over the 48 head rows via a DRAM round-trip
                dscr = dram.tile([64, L], F32, name=f"dscr{m}", tag=f"dscr{m}")
                nc.sync.dma_start(out=dscr[:, :], in_=denom)
                bcD = nrm.tile([128, L], F32, name="bcD", tag="bcD")
                bc_src = bass.AP(
                    tensor=dscr.tensor, offset=dscr.offset,
                    ap=[[32 * L, 2], [0, 64], [1, L]],
                )
                nc.sync.dma_start(out=bcD, in_=bc_src)
                nc.vector.tensor_mul(out=OT_sb[m], in0=OT_sb[m], in1=bcD)
                # V bias (usually zero): per-partition add
                nc.vector.tensor_scalar(
                    out=OT_sb[m], in0=OT_sb[m], scalar1=vbias_sb[:, m : m + 1],
                    scalar2=None, op0=ALU.add,
                )

            # ---- pre branch: gelu(tanh approx) ----
            G_sb = [work.tile([128, L], BF16, name=f"G{m}", tag=f"G{m}") for m in range(DC)]
            for m in range(DC):
                for tp in range(2):
                    ps = psA.tile([128, 1024], F32, name="ps_pre", tag="psA")
                    for half in range(2):
                        tt = 2 * tp + half
                        for c in range(DC):
                            nc.tensor.matmul(
                                out=ps[:, QT * half : QT * (half + 1)],
                                lhsT=w_sb[c][:, PCOL + 128 * m : PCOL + 128 * (m + 1)],
                                rhs=hT_sb[c][:, QT * tt : QT * (tt + 1)],
                                start=(c == 0), stop=(c == DC - 1),
                            )
                    nc.scalar.activation(
                        out=G_sb[m][:, 1024 * tp : 1024 * (tp + 1)], in_=ps,
                        func=gelu, bias=preb_sb[:, m : m + 1], scale=1.0,
                    )

            # ---- exchange o-halves within batch pair (AllGather) ----
            oswap_in = dram.tile([256, L], BF16, name="oswap_in", tag="oswap_in")
            oswap_out = dram.tile([512, L], BF16, name="oswap_out", tag="oswap_out")
            for m in range(2):
                nc.sync.dma_start(out=oswap_in[128 * m : 128 * (m + 1), :], in_=OT_sb[m])
            nc.gpsimd.collective_compute(
                "AllGather", ALU.bypass,
                replica_groups=[[0, 1], [2, 3], [4, 5], [6, 7]],
                ins=[oswap_in.opt()], outs=[oswap_out.opt()],
            )
            catO = []
            for i in range(4):
                t = work.tile([128, L], BF16, name=f"catO{i}", tag=f"catO{i}")
                nc.sync.dma_start(out=t, in_=oswap_out[128 * i : 128 * (i + 1), :])
                catO.append(t)

            # ---- out projection + residual ----
            cat_tiles = G_sb + catO  # 7 chunks matching outwT order
            for t16 in range(16):
                po = psB.tile([128, D], F32, name="ps_out", tag="psB")
                for i in range(7):
                    nc.tensor.matmul(
                        out=po,
                        lhsT=cat_tiles[i][:, 128 * t16 : 128 * (t16 + 1)],
                        rhs=outw_sb[i],
                        start=(i == 0), stop=False,
                    )
                nc.tensor.matmul(
                    out=po,
                    lhsT=ones_sb[:, 128 * t16 : 128 * (t16 + 1)],
                    rhs=outb_sb,
                    start=False, stop=True,
                )
                ro = outpool.tile([128, D], F32, name="ro", tag="ro")
                nc.vector.tensor_add(out=ro, in0=po, in1=xres_sb[t16])
                nc.sync.dma_start(out=out[128 * t16 : 128 * (t16 + 1), :], in_=ro)

    if split_waits:
        _split_multi_waits(nc)
    return nc


def _split_multi_waits(nc):
    """This toolchain's walrus encodes at most one sync-wait per instruction;
    hoist extra waits into standalone EventSemaphore instructions on the same
    engine immediately before the original instruction."""
    for bb in nc.main_func.blocks:
        insts = list(bb.instructions)
        if not any(
            ins.sync_info is not None and len(ins.sync_info.on_wait) > 1
            for ins in insts
        ):
            continue
        new = []
        for ins in insts:
            si = ins.sync_info
            if si is not None and len(si.on_wait) > 1:
                waits = list(si.on_wait)
                for k, w in enumerate(waits[:-1]):
                    es = mybir.InstEventSemaphore(name=f"{ins.name}-w{k}", ins=[], outs=[])
                    es.engine = ins.engine
                    es.sync_info = _bass_rust.SyncInfo(on_wait=[w], on_update=[])
                    new.append(es)
                ins.sync_info = _bass_rust.SyncInfo(
                    on_wait=[waits[-1]], on_update=list(si.on_update)
                )
            new.append(ins)
        bb.instructions = new


def _prep_core_inputs(x, norm_gamma, norm_beta, qkv_w, qkv_b, out_w, out_b, core):
    bf16 = ml_dtypes.bfloat16
    b, s = core // 2, core % 2
    heads = [4 * s + i for i in range(H_LOC)]

    # xT: own batch first, remaining batches after (stats are order-invariant)
    other = [bb for bb in range(B) if bb != b]
    xt_cols = [x[b].T] + [x[bb].T for bb in other]
    xT = np.ascontiguousarray(np.concatenate(xt_cols, axis=1)).astype(bf16)

    xres = np.ascontiguousarray(x[b]).astype(np.float32)

    gamma = np.ascontiguousarray(norm_gamma.reshape(DC, 128).T).astype(np.float32)
    beta = np.ascontiguousarray(norm_beta.reshape(DC, 128).T).astype(np.float32)

    wq, wk, wv, wpre = (qkv_w[i * D : (i + 1) * D] for i in range(4))
    bq, bk, bv, bpre = (qkv_b[i * D : (i + 1) * D] for i in range(4))

    qkvwT = np.zeros((D, QKVW_COLS), np.float32)
    qkvb_arr = np.zeros((128, 4), np.float32)
    for m in range(2):
        for hh in range(2):
            hglob = heads[2 * m + hh]
            rows = slice(HD * hglob, HD * (hglob + 1))
            o0 = 64 * hh
            qkvwT[:, QCOL + 128 * m + o0 : QCOL + 128 * m + o0 + HD] = wq[rows].T
            qkvwT[:, KCOL + 128 * m + o0 : KCOL + 128 * m + o0 + HD] = wk[rows].T
            qkvb_arr[o0 : o0 + HD, m] = bq[rows]
            qkvb_arr[o0 : o0 + HD, 2 + m] = bk[rows]
    qkvwT[:, VCOL : VCOL + H_LOC * HD] = wv[HD * heads[0] : HD * (heads[-1] + 1)].T
    qkvwT[:, PCOL : PCOL + D] = wpre.T
    qkvwT = qkvwT.astype(bf16)

    preb = np.ascontiguousarray(bpre.reshape(DC, 128).T).astype(np.float32)
    vb = np.zeros((128, 2), np.float32)
    for m in range(2):
        for hh in range(2):
            hglob = heads[2 * m + hh]
            vb[64 * hh : 64 * hh + HD, m] = bv[HD * hglob : HD * (hglob + 1)]

    # out projection: out_w [D, 768]; rows 0:384 gelu(pre) dims, 384:768 o dims
    owT = out_w.T.astype(np.float32)  # [768, D]
    outwT = np.zeros((7, 128, D), np.float32)
    for m in range(DC):
        outwT[m] = owT[128 * m : 128 * (m + 1)]
    for p2 in range(4):  # o chunks, global head order, heads at row offsets 0/64
        outwT[DC + p2, 0:HD, :] = owT[D + HD * 2 * p2 : D + HD * (2 * p2 + 1)]
        outwT[DC + p2, 64 : 64 + HD, :] = owT[D + HD * (2 * p2 + 1) : D + HD * (2 * p2 + 2)]
    outwT = outwT.astype(bf16)
    outb_arr = out_b.reshape(1, D).astype(bf16)

    # causal masks for diagonal k-tile pairs: mask_dd[kl, ql] = ql >= kl + 128*dd
    ql = np.arange(QT)[None, :]
    kl = np.arange(128)[:, None]
    m4 = [(ql >= kl + 128 * dd).astype(np.float32) for dd in range(4)]
    masks = np.zeros((2, 128, 1024), np.float32)
    masks[0, :, 0:QT] = m4[0]
    masks[0, :, QT:1024] = m4[1]
    masks[1, :, 0:QT] = m4[2]
    masks[1, :, QT:1024] = m4[3]
    masks = masks.astype(bf16)

    ones_arr = np.ones((1, L), bf16)

    return {
        "xT": xT, "xres": xres, "gamma": gamma, "beta": beta,
        "qkvwT": qkvwT, "qkvb": qkvb_arr, "preb": preb, "vbias": vb,
        "outwT": outwT, "outb": outb_arr, "masks": masks, "ones_row": ones_arr,
    }


def _install_ntff_shim():
    """Provide antenv.axon_hooks (absent in this image) so bass_utils'
    trace path can reach the axon NTFF profiler via ctypes."""
    try:
        import sys, types
        import antenv
        if "antenv.axon_hooks" not in sys.modules:
            from trn_agent_boot.trn_boot import _ntff_profile_via_ctypes
            hook = _ntff_profile_via_ctypes("/opt/axon/libaxon_pjrt.so")
            mod = types.ModuleType("antenv.axon_hooks")
            mod._hook = hook
            mod.set_axon_ntff_profile_hook = lambda h: setattr(mod, "_hook", h)
            mod.get_axon_ntff_profile_hook = lambda: mod._hook
            sys.modules["antenv.axon_hooks"] = mod
            antenv.axon_hooks = mod
        import concourse.bass_utils as _bu
        _bu.upload_artifacts = lambda d: "local"
        return True
    except Exception as e:
        print(f"ntff shim unavailable: {e!r}")
        return False


def kernel(x, norm_gamma, norm_beta, qkv_w, qkv_b, out_w, out_b):
    global LAST_EXEC_TIME_NS, LAST_RESULTS
    x = np.asarray(x, np.float32)
    norm_gamma = np.asarray(norm_gamma, np.float32)
    norm_beta = np.asarray(norm_beta, np.float32)
    qkv_w = np.asarray(qkv_w, np.float32)
    qkv_b = np.asarray(qkv_b, np.float32)
    out_w = np.asarray(out_w, np.float32)
    out_b = np.asarray(out_b, np.float32)

    if "nc" not in _PROGRAM_CACHE:
        _PROGRAM_CACHE["nc"] = _build_program()
    nc = _PROGRAM_CACHE["nc"]

    in_maps = [
        _prep_core_inputs(x, norm_gamma, norm_beta, qkv_w, qkv_b, out_w, out_b, c)
        for c in range(N_CORES)
    ]
    trace = os.environ.get("KERNEL_TRACE", "0") == "1"
    if trace:
        trace = _install_ntff_shim()
    res = run_bass_kernel_spmd(
        nc, in_maps, list(range(N_CORES)), trace=trace,
        trace_cores=list(range(N_CORES)) if trace else None,
    )
    LAST_EXEC_TIME_NS = res.exec_time_ns
    LAST_RESULTS = res
    out = np.empty((B, L, D), np.float32)
    for b in range(B):
        out[b] = res.results[2 * b]["out"]
    return out


# revision 19
# speedup vs baseline: 1.0665x; 1.0665x over previous
"""Trainium2 Bass kernel for nn_Block (BatchNorm -> QKV -> causal MHA + gelu gate -> out proj + residual).

Contract: kernel(**inputs) takes FULL unsharded inputs (np arrays, keys as in
setup_inputs()) and returns the FULL output (4, 2048, 384) float32.

Sharding: 8 cores; core c handles batch b=c//2 and head-half s=c%2 (4 of 8 heads).
Both cores of a batch pair compute the full residual output for their batch
(symmetric SPMD program); o-halves are exchanged with a 2-rank AllGather.
BatchNorm statistics are computed replicated on every core from a
column-permuted transposed copy of the full x (own batch's 2048 columns first,
so the program is identical across cores).
"""

import math
import os

import numpy as np
import ml_dtypes

import bass_rust as _bass_rust
import concourse.bass as bass
import concourse.tile as tile
import concourse.mybir as mybir
from concourse.bass_utils import run_bass_kernel_spmd

BF16 = mybir.dt.bfloat16
F32 = mybir.dt.float32
AF = mybir.ActivationFunctionType
ALU = mybir.AluOpType

# Problem constants (hardcoded per harness contract)
B, L, D = 4, 2048, 384
HEADS = 8
HD = 48  # head dim
H_LOC = 4  # heads per core
EXPAND = 2
EPS = 1e-5
NTOK = B * L  # tokens for batchnorm stats
N_CORES = 8
QT = 512  # q tile width
NQT = L // QT  # 4 q tiles
NKT = L // 128  # 16 k tiles
DC = D // 128  # 3 d-chunks
SCALE = 1.0 / math.sqrt(HD)

# qkvwT packed column layout: [Qpack 256 | Kpack 256 | V 192 | pre 384]
QCOL, KCOL, VCOL, PCOL = 0, 256, 512, 704
QKVW_COLS = 1088

LAST_EXEC_TIME_NS = None
LAST_RESULTS = None

_PROGRAM_CACHE = {}


def _build_program(split_waits=True, gelu_func=None):
    gelu = gelu_func or AF.Gelu_apprx_tanh
    nc = bass.Bass()

    # ---- I/O ----
    xT = nc.dram_tensor("xT", [D, NTOK], BF16, kind="ExternalInput")
    xres = nc.dram_tensor("xres", [L, D], F32, kind="ExternalInput")
    gamma = nc.dram_tensor("gamma", [128, DC], F32, kind="ExternalInput")
    beta = nc.dram_tensor("beta", [128, DC], F32, kind="ExternalInput")
    qkvwT = nc.dram_tensor("qkvwT", [D, QKVW_COLS], BF16, kind="ExternalInput")
    qkvb = nc.dram_tensor("qkvb", [128, 4], F32, kind="ExternalInput")
    preb = nc.dram_tensor("preb", [128, DC], F32, kind="ExternalInput")
    vbias = nc.dram_tensor("vbias", [128, 2], F32, kind="ExternalInput")
    outwT = nc.dram_tensor("outwT", [7, 128, D], BF16, kind="ExternalInput")
    outb = nc.dram_tensor("outb", [1, D], BF16, kind="ExternalInput")
    masks = nc.dram_tensor("masks", [2, 128, 1024], BF16, kind="ExternalInput")
    ones_row = nc.dram_tensor("ones_row", [1, L], BF16, kind="ExternalInput")
    out = nc.dram_tensor("out", [L, D], F32, kind="ExternalOutput")

    with tile.TileContext(nc) as tc:
        with (
            tc.tile_pool(name="const", bufs=1) as const,
            tc.tile_pool(name="work", bufs=1) as work,
            tc.tile_pool(name="ptp", bufs=3) as ptp,
            tc.tile_pool(name="scr", bufs=2) as scr,
            tc.tile_pool(name="outp", bufs=3) as outpool,
            tc.tile_pool(name="nrm", bufs=1) as nrm,
            tc.tile_pool(name="psA", bufs=2, space="PSUM") as psA,
            tc.tile_pool(name="psB", bufs=2, space="PSUM") as psB,
            tc.tile_pool(name="psO", bufs=2, space="PSUM") as psO,
            tc.tile_pool(name="dram", bufs=1, space="DRAM") as dram,
        ):
            # ---- load inputs to SBUF ----
            xT_sb = []
            for c in range(DC):
                t = const.tile([128, NTOK], BF16, name=f"xT_sb{c}", tag=f"xT_sb{c}")
                for i in range(4):
                    w = NTOK // 4
                    nc.sync.dma_start(
                        out=t[:, w * i : w * (i + 1)],
                        in_=xT[128 * c : 128 * (c + 1), w * i : w * (i + 1)],
                    )
                xT_sb.append(t)
            xres_sb = []
            for t16 in range(16):
                t = const.tile([128, D], F32, name=f"xres{t16}", tag=f"xres{t16}")
                nc.sync.dma_start(out=t, in_=xres[128 * t16 : 128 * (t16 + 1), :])
                xres_sb.append(t)
            w_sb = []
            for c in range(DC):
                t = const.tile([128, QKVW_COLS], BF16, name=f"w{c}", tag=f"w{c}")
                nc.sync.dma_start(out=t, in_=qkvwT[128 * c : 128 * (c + 1), :])
                w_sb.append(t)
            outw_sb = []
            for i in range(7):
                t = const.tile([128, D], BF16, name=f"outw{i}", tag=f"outw{i}")
                nc.sync.dma_start(out=t, in_=outwT[i])
                outw_sb.append(t)
            outb_sb = const.tile([1, D], BF16, name="outb_sb", tag="outb_sb")
            nc.sync.dma_start(out=outb_sb, in_=outb[:, :])
            masks_sb = []
            for i in range(2):
                t = const.tile([128, 1024], BF16, name=f"mask{i}", tag=f"mask{i}")
                nc.sync.dma_start(out=t, in_=masks[i])
                masks_sb.append(t)
            ones_sb = const.tile([1, L], BF16, name="ones_sb", tag="ones_sb")
            nc.sync.dma_start(out=ones_sb, in_=ones_row[:, :])
            gamma_sb = const.tile([128, DC], F32, name="gamma_sb", tag="gamma_sb")
            nc.sync.dma_start(out=gamma_sb, in_=gamma[:, :])
            beta_sb = const.tile([128, DC], F32, name="beta_sb", tag="beta_sb")
            nc.sync.dma_start(out=beta_sb, in_=beta[:, :])
            qkvb_sb = const.tile([128, 4], F32, name="qkvb_sb", tag="qkvb_sb")
            nc.sync.dma_start(out=qkvb_sb, in_=qkvb[:, :])
            preb_sb = const.tile([128, DC], F32, name="preb_sb", tag="preb_sb")
            nc.sync.dma_start(out=preb_sb, in_=preb[:, :])
            vbias_sb = const.tile([128, 2], F32, name="vbias_sb", tag="vbias_sb")
            nc.sync.dma_start(out=vbias_sb, in_=vbias[:, :])

            # ---- batchnorm statistics (replicated over full x) ----
            # per d-chunk independent stats so hT_c unblocks as soon as its
            # chunk is reduced; chunk 1 runs on ScalarE to parallelize with DVE
            NCH = NTOK // 512
            eps_sb = work.tile([128, 1], F32, name="eps_sb", tag="eps_sb")
            nc.vector.memset(eps_sb, EPS)
            s_sb = work.tile([128, DC], F32, name="s_sb", tag="s_sb")
            bb_sb = work.tile([128, DC], F32, name="bb_sb", tag="bb_sb")
            for c in range(DC):
                mv_c = work.tile([128, 2], F32, name=f"mv{c}", tag=f"mv{c}")
                if c == 1:
                    acc = work.tile([128, 2], F32, name=f"acc{c}", tag=f"acc{c}")
                    nc.scalar.activation(out=xT_sb[c], in_=xT_sb[c], func=AF.Copy,
                                         scale=1.0, accum_out=acc[:, 0:1])
                    sc2 = scr.tile([128, NTOK], BF16, name="actscr", tag="actscr",
                                   bufs=1)
                    nc.scalar.activation(out=sc2, in_=xT_sb[c], func=AF.Square,
                                         scale=1.0, accum_out=acc[:, 1:2])
                    # mean = sum/N ; var = sumsq/N - mean^2
                    nc.scalar.activation(out=mv_c[:, 0:1], in_=acc[:, 0:1],
                                         func=AF.Copy, scale=1.0 / NTOK)
                    ex2 = work.tile([128, 1], F32, name=f"ex2{c}", tag=f"ex2{c}")
                    nc.scalar.activation(out=ex2, in_=acc[:, 1:2],
                                         func=AF.Copy, scale=1.0 / NTOK)
                    nc.vector.scalar_tensor_tensor(
                        out=mv_c[:, 1:2], in0=mv_c[:, 0:1], scalar=-1.0,
                        in1=mv_c[:, 0:1], op0=ALU.mult, op1=ALU.mult)
                    nc.vector.tensor_add(out=mv_c[:, 1:2], in0=mv_c[:, 1:2], in1=ex2)
                else:
                    st6 = work.tile([128, NCH, 6], F32, name=f"st6_{c}", tag=f"st6_{c}")
                    for i in range(NCH):
                        nc.vector.bn_stats(
                            out=st6[:, i, :], in_=xT_sb[c][:, 512 * i : 512 * (i + 1)]
                        )
                    nc.vector.bn_aggr(out=mv_c, in_=st6)
                # s = rstd*gamma (rstd = exp(-0.5*ln(var+eps))); bb = beta - mean*s
                lnv = work.tile([128, 1], F32, name=f"lnv{c}", tag=f"lnv{c}")
                nc.scalar.activation(out=lnv, in_=mv_c[:, 1:2], func=AF.Ln,
                                     bias=eps_sb, scale=1.0)
                rstd = work.tile([128, 1], F32, name=f"rstd{c}", tag=f"rstd{c}")
                nc.scalar.activation(out=rstd, in_=lnv, func=AF.Exp, scale=-0.5)
                nc.vector.tensor_mul(out=s_sb[:, c : c + 1], in0=rstd,
                                     in1=gamma_sb[:, c : c + 1])
                nc.vector.scalar_tensor_tensor(
                    out=bb_sb[:, c : c + 1], in0=mv_c[:, 0:1], scalar=-1.0,
                    in1=s_sb[:, c : c + 1], op0=ALU.mult, op1=ALU.mult)
                nc.vector.tensor_add(out=bb_sb[:, c : c + 1],
                                     in0=bb_sb[:, c : c + 1], in1=beta_sb[:, c : c + 1])

            # ---- normalize own batch -> hT (bf16) ----
            hT_sb = []
            for c in range(DC):
                t = work.tile([128, L], BF16, name=f"hT{c}", tag=f"hT{c}")
                nc.vector.tensor_scalar(
                    out=t, in0=xT_sb[c][:, 0:L], scalar1=s_sb[:, c : c + 1],
                    scalar2=bb_sb[:, c : c + 1], op0=ALU.mult, op1=ALU.add,
                )
                hT_sb.append(t)

            # ---- QKV projections ----
            # QT/KT: transposed [outch, tok]; heads packed 2-per-tile (48+48+32pad)
            QT_sb = [work.tile([128, L], BF16, name=f"QTt{m}", tag=f"QTt{m}") for m in range(2)]
            KT_sb = [work.tile([128, L], BF16, name=f"KTt{m}", tag=f"KTt{m}") for m in range(2)]
            for dst, col0, bcol0 in ((QT_sb, QCOL, 0), (KT_sb, KCOL, 2)):
                for m in range(2):
                    for tt in range(NQT):
                        ps = psB.tile([128, QT], F32, name="ps_qk", tag="psB")
                        for c in range(DC):
                            nc.tensor.matmul(
                                out=ps,
                                lhsT=w_sb[c][:, col0 + 128 * m : col0 + 128 * (m + 1)],
                                rhs=hT_sb[c][:, QT * tt : QT * (tt + 1)],
                                start=(c == 0), stop=(c == DC - 1),
                            )
                        nc.vector.tensor_scalar(
                            out=dst[m][:, QT * tt : QT * (tt + 1)], in0=ps,
                            scalar1=qkvb_sb[:, bcol0 + m : bcol0 + m + 1], scalar2=None,
                            op0=ALU.add,
                        )
            # V natural [tok, 4 heads * 49] with ones column per head
            V_sb = []
            for t16 in range(16):
                vt = work.tile([128, H_LOC * 65], BF16, name=f"V{t16}", tag=f"V{t16}")
                nc.vector.memset(vt, 1.0)
                V_sb.append(vt)
            for t16 in range(16):
                ps = psB.tile([128, H_LOC * HD], F32, name="ps_v", tag="psB")
                for c in range(DC):
                    nc.tensor.matmul(
                        out=ps,
                        lhsT=hT_sb[c][:, 128 * t16 : 128 * (t16 + 1)],
                        rhs=w_sb[c][:, VCOL : VCOL + H_LOC * HD],
                        start=(c == 0), stop=(c == DC - 1),
                    )
                dstv = V_sb[t16].rearrange("p (h d) -> p h d", h=H_LOC)[:, :, 0:HD]
                nc.vector.tensor_copy(
                    out=dstv, in_=ps.rearrange("p (h d) -> p h d", h=H_LOC)
                )

            # ---- attention (4 local heads, causal) ----
            catO = [None] * 4
            OT_sb = [work.tile([128, L], BF16, name=f"OTt{m}", tag=f"OTt{m}") for m in range(2)]
            for m in range(2):
                nc.vector.memset(OT_sb[m], 0.0)
            for m in range(2):
                denom = nrm.tile([64, L], F32, name="denom", tag="denom")
                nc.vector.memset(denom, 0.0)
                for hh in range(2):
                    h, ko = 2 * m + hh, 64 * hh
                    for j in range(NQT):
                        ext = 4 * (j + 1)  # k tiles for this q tile
                        ot = psO.tile([65, QT], F32, name="ps_ot", tag="psO")
                        for p in range(ext // 2):
                            st = psA.tile([128, 1024], F32, name="ps_st", tag="psA")
                            for half in range(2):
                                t = 2 * p + half
                                nc.tensor.matmul(
                                    out=st[:, QT * half : QT * (half + 1)],
                                    lhsT=KT_sb[m][ko : ko + HD, 128 * t : 128 * (t + 1)],
                                    rhs=QT_sb[m][ko : ko + HD, QT * j : QT * (j + 1)],
                                    start=True, stop=True,
                                )
                            pt = ptp.tile([128, 1024], BF16, name="pt", tag="pt")
                            nc.scalar.activation(out=pt, in_=st, func=AF.Exp, scale=SCALE)
                            if p == 2 * j:
                                nc.vector.tensor_mul(out=pt, in0=pt, in1=masks_sb[0])
                            elif p == 2 * j + 1:
                                nc.vector.tensor_mul(out=pt, in0=pt, in1=masks_sb[1])
                            for half in range(2):
                                t = 2 * p + half
                                nc.tensor.matmul(
                                    out=ot,
                                    lhsT=V_sb[t][:, 65 * h : 65 * (h + 1)],
                                    rhs=pt[:, QT * half : QT * (half + 1)],
                                    start=(p == 0 and half == 0),
                                    stop=(p == ext // 2 - 1 and half == 1),
                                )
                        # softmax denominator (row 64) copied off on ScalarE;
                        # unnormalized o copied on DVE
                        nc.scalar.activation(
                            out=denom[32 * hh : 32 * hh + 1, QT * j : QT * (j + 1)],
                            in_=ot[64:65, :], func=AF.Copy, scale=1.0,
                        )
                        nc.vector.tensor_copy(
                            out=OT_sb[m][ko : ko + HD, QT * j : QT * (j + 1)],
                            in_=ot[0:HD, :],
                        )
                # one batched reciprocal, then broadcast over the 48 head rows
                # via a DRAM round-trip
                nc.vector.reciprocal(out=denom, in_=denom)
                dscr = dram.tile([64, L], F32, name=f"dscr{m}", tag=f"dscr{m}")
                nc.sync.dma_start(out=dscr[:, :], in_=denom)
                bcD = nrm.tile([128, L], F32, name="bcD", tag="denom")
                bc_src = bass.AP(
                    tensor=dscr.tensor, offset=dscr.offset,
                    ap=[[32 * L, 2], [0, 64], [1, L]],
                )
                nc.sync.dma_start(out=bcD, in_=bc_src)
                nc.vector.tensor_mul(out=OT_sb[m], in0=OT_sb[m], in1=bcD)
                # V bias (usually zero): per-partition add
                nc.vector.tensor_scalar(
                    out=OT_sb[m], in0=OT_sb[m], scalar1=vbias_sb[:, m : m + 1],
                    scalar2=None, op0=ALU.add,
                )
                # exchange this head-pair within the batch pair (AllGather);
                # m=0's collective overlaps m=1's attention
                oswap_in = dram.tile([128, L], BF16, name=f"oswap_in{m}",
                                     tag=f"oswap_in{m}")
                oswap_out = dram.tile([256, L], BF16, name=f"oswap_out{m}",
                                      tag=f"oswap_out{m}")
                nc.sync.dma_start(out=oswap_in[:, :], in_=OT_sb[m])
                nc.gpsimd.collective_compute(
                    "AllGather", ALU.bypass,
                    replica_groups=[[0, 1], [2, 3], [4, 5], [6, 7]],
                    ins=[oswap_in.opt()], outs=[oswap_out.opt()],
                )
                # AG_m blocks: [my pair-m heads | partner pair-m heads] ->
                # global head order: catO[m] = blk0, catO[m+2] = blk1
                t = work.tile([128, L], BF16, name=f"catO{m}", tag=f"catO{m}")
                nc.sync.dma_start(out=t, in_=oswap_out[0:128, :])
                catO[m] = t
                t2 = work.tile([128, L], BF16, name=f"catO{m + 2}", tag=f"catO{m + 2}")
                nc.sync.dma_start(out=t2, in_=oswap_out[128:256, :])
                catO[m + 2] = t2

            # ---- pre branch: gelu(tanh approx) ----
            G_sb = [work.tile([128, L], BF16, name=f"G{m}", tag=f"G{m}") for m in range(DC)]
            for m in range(DC):
                for tp in range(2):
                    ps = psA.tile([128, 1024], F32, name="ps_pre", tag="psA")
                    for half in range(2):
                        tt = 2 * tp + half
                        for c in range(DC):
                            nc.tensor.matmul(
                                out=ps[:, QT * half : QT * (half + 1)],
                                lhsT=w_sb[c][:, PCOL + 128 * m : PCOL + 128 * (m + 1)],
                                rhs=hT_sb[c][:, QT * tt : QT * (tt + 1)],
                                start=(c == 0), stop=(c == DC - 1),
                            )
                    nc.scalar.activation(
                        out=G_sb[m][:, 1024 * tp : 1024 * (tp + 1)], in_=ps,
                        func=gelu, bias=preb_sb[:, m : m + 1], scale=1.0,
                    )

            # ---- out projection + residual ----
            cat_tiles = G_sb + catO  # 7 chunks matching outwT order
            for t16 in range(16):
                po = psB.tile([128, D], F32, name="ps_out", tag="psB")
                for i in range(7):
                    nc.tensor.matmul(
                        out=po,
                        lhsT=cat_tiles[i][:, 128 * t16 : 128 * (t16 + 1)],
                        rhs=outw_sb[i],
                        start=(i == 0), stop=False,
                    )
                nc.tensor.matmul(
                    out=po,
                    lhsT=ones_sb[:, 128 * t16 : 128 * (t16 + 1)],
                    rhs=outb_sb,
                    start=False, stop=True,
                )
                ro = outpool.tile([128, D], F32, name="ro", tag="ro")
                nc.vector.tensor_add(out=ro, in0=po, in1=xres_sb[t16])
                nc.sync.dma_start(out=out[128 * t16 : 128 * (t16 + 1), :], in_=ro)

    if split_waits:
        _split_multi_waits(nc)
    return nc


def _split_multi_waits(nc):
    """This toolchain's walrus encodes at most one sync-wait per instruction;
    hoist extra waits into standalone EventSemaphore instructions on the same
    engine immediately before the original instruction."""
    for bb in nc.main_func.blocks:
        insts = list(bb.instructions)
        if not any(
            ins.sync_info is not None and len(ins.sync_info.on_wait) > 1
            for ins in insts
        ):
            continue
        new = []
        for ins in insts:
            si = ins.sync_info
            if si is not None and len(si.on_wait) > 1:
                waits = list(si.on_wait)
                for k, w in enumerate(waits[:-1]):
                    es = mybir.InstEventSemaphore(name=f"{ins.name}-w{k}", ins=[], outs=[])
                    es.engine = ins.engine
                    es.sync_info = _bass_rust.SyncInfo(on_wait=[w], on_update=[])
                    new.append(es)
                ins.sync_info = _bass_rust.SyncInfo(
                    on_wait=[waits[-1]], on_update=list(si.on_update)
                )
            new.append(ins)
        bb.instructions = new


def _prep_core_inputs(x, norm_gamma, norm_beta, qkv_w, qkv_b, out_w, out_b, core):
    bf16 = ml_dtypes.bfloat16
    b, s = core // 2, core % 2
    heads = [4 * s + i for i in range(H_LOC)]

    # xT: own batch first, remaining batches after (stats are order-invariant)
    other = [bb for bb in range(B) if bb != b]
    xt_cols = [x[b].T] + [x[bb].T for bb in other]
    xT = np.ascontiguousarray(np.concatenate(xt_cols, axis=1)).astype(bf16)

    xres = np.ascontiguousarray(x[b]).astype(np.float32)

    gamma = np.ascontiguousarray(norm_gamma.reshape(DC, 128).T).astype(np.float32)
    beta = np.ascontiguousarray(norm_beta.reshape(DC, 128).T).astype(np.float32)

    wq, wk, wv, wpre = (qkv_w[i * D : (i + 1) * D] for i in range(4))
    bq, bk, bv, bpre = (qkv_b[i * D : (i + 1) * D] for i in range(4))

    qkvwT = np.zeros((D, QKVW_COLS), np.float32)
    qkvb_arr = np.zeros((128, 4), np.float32)
    for m in range(2):
        for hh in range(2):
            hglob = heads[2 * m + hh]
            rows = slice(HD * hglob, HD * (hglob + 1))
            o0 = 64 * hh
            qkvwT[:, QCOL + 128 * m + o0 : QCOL + 128 * m + o0 + HD] = wq[rows].T
            qkvwT[:, KCOL + 128 * m + o0 : KCOL + 128 * m + o0 + HD] = wk[rows].T
            qkvb_arr[o0 : o0 + HD, m] = bq[rows]
            qkvb_arr[o0 : o0 + HD, 2 + m] = bk[rows]
    qkvwT[:, VCOL : VCOL + H_LOC * HD] = wv[HD * heads[0] : HD * (heads[-1] + 1)].T
    qkvwT[:, PCOL : PCOL + D] = wpre.T
    qkvwT = qkvwT.astype(bf16)

    preb = np.ascontiguousarray(bpre.reshape(DC, 128).T).astype(np.float32)
    vb = np.zeros((128, 2), np.float32)
    for m in range(2):
        for hh in range(2):
            hglob = heads[2 * m + hh]
            vb[64 * hh : 64 * hh + HD, m] = bv[HD * hglob : HD * (hglob + 1)]

    # out projection: out_w [D, 768]; rows 0:384 gelu(pre) dims, 384:768 o dims
    owT = out_w.T.astype(np.float32)  # [768, D]
    outwT = np.zeros((7, 128, D), np.float32)
    for m in range(DC):
        outwT[m] = owT[128 * m : 128 * (m + 1)]
    for p2 in range(4):  # o chunks, global head order, heads at row offsets 0/64
        outwT[DC + p2, 0:HD, :] = owT[D + HD * 2 * p2 : D + HD * (2 * p2 + 1)]
        outwT[DC + p2, 64 : 64 + HD, :] = owT[D + HD * (2 * p2 + 1) : D + HD * (2 * p2 + 2)]
    outwT = outwT.astype(bf16)
    outb_arr = out_b.reshape(1, D).astype(bf16)

    # causal masks for diagonal k-tile pairs: mask_dd[kl, ql] = ql >= kl + 128*dd
    ql = np.arange(QT)[None, :]
    kl = np.arange(128)[:, None]
    m4 = [(ql >= kl + 128 * dd).astype(np.float32) for dd in range(4)]
    masks = np.zeros((2, 128, 1024), np.float32)
    masks[0, :, 0:QT] = m4[0]
    masks[0, :, QT:1024] = m4[1]
    masks[1, :, 0:QT] = m4[2]
    masks[1, :, QT:1024] = m4[3]
    masks = masks.astype(bf16)

    ones_arr = np.ones((1, L), bf16)

    return {
        "xT": xT, "xres": xres, "gamma": gamma, "beta": beta,
        "qkvwT": qkvwT, "qkvb": qkvb_arr, "preb": preb, "vbias": vb,
        "outwT": outwT, "outb": outb_arr, "masks": masks, "ones_row": ones_arr,
    }


def _install_ntff_shim():
    """Provide antenv.axon_hooks (absent in this image) so bass_utils'
    trace path can reach the axon NTFF profiler via ctypes."""
    try:
        import sys, types
        import antenv
        if "antenv.axon_hooks" not in sys.modules:
            from trn_agent_boot.trn_boot import _ntff_profile_via_ctypes
            hook = _ntff_profile_via_ctypes("/opt/axon/libaxon_pjrt.so")
            mod = types.ModuleType("antenv.axon_hooks")
            mod._hook = hook
            mod.set_axon_ntff_profile_hook = lambda h: setattr(mod, "_hook", h)
            mod.get_axon_ntff_profile_hook = lambda: mod._hook
            sys.modules["antenv.axon_hooks"] = mod
            antenv.axon_hooks = mod
        import concourse.bass_utils as _bu
        _bu.upload_artifacts = lambda d: "local"
        return True
    except Exception as e:
        print(f"ntff shim unavailable: {e!r}")
        return False


def kernel(x, norm_gamma, norm_beta, qkv_w, qkv_b, out_w, out_b):
    global LAST_EXEC_TIME_NS, LAST_RESULTS
    x = np.asarray(x, np.float32)
    norm_gamma = np.asarray(norm_gamma, np.float32)
    norm_beta = np.asarray(norm_beta, np.float32)
    qkv_w = np.asarray(qkv_w, np.float32)
    qkv_b = np.asarray(qkv_b, np.float32)
    out_w = np.asarray(out_w, np.float32)
    out_b = np.asarray(out_b, np.float32)

    if "nc" not in _PROGRAM_CACHE:
        _PROGRAM_CACHE["nc"] = _build_program()
    nc = _PROGRAM_CACHE["nc"]

    in_maps = [
        _prep_core_inputs(x, norm_gamma, norm_beta, qkv_w, qkv_b, out_w, out_b, c)
        for c in range(N_CORES)
    ]
    trace = os.environ.get("KERNEL_TRACE", "0") == "1"
    if trace:
        trace = _install_ntff_shim()
    res = run_bass_kernel_spmd(
        nc, in_maps, list(range(N_CORES)), trace=trace,
        trace_cores=list(range(N_CORES)) if trace else None,
    )
    LAST_EXEC_TIME_NS = res.exec_time_ns
    LAST_RESULTS = res
    out = np.empty((B, L, D), np.float32)
    for b in range(B):
        out[b] = res.results[2 * b]["out"]
    return out


# revision 30
# speedup vs baseline: 1.2295x; 1.1528x over previous
"""Trainium2 Bass kernel for nn_Block (BatchNorm -> QKV -> causal MHA + gelu gate -> out proj + residual).

Contract: kernel(**inputs) takes FULL unsharded inputs (np arrays, keys as in
setup_inputs()) and returns the FULL output (4, 2048, 384) float32.

Sharding: 8 cores; core c handles batch b=c//2 and head-half s=c%2 (4 of 8 heads).
Both cores of a batch pair compute the full residual output for their batch
(symmetric SPMD program); o-halves are exchanged with a 2-rank AllGather.
BatchNorm statistics are computed replicated on every core from a
column-permuted transposed copy of the full x (own batch's 2048 columns first,
so the program is identical across cores).
"""

import math
import os

import numpy as np
import ml_dtypes

import bass_rust as _bass_rust
import concourse.bass as bass
import concourse.tile as tile
import concourse.mybir as mybir
from concourse.bass_utils import run_bass_kernel_spmd

BF16 = mybir.dt.bfloat16
F32 = mybir.dt.float32
AF = mybir.ActivationFunctionType
ALU = mybir.AluOpType

# Problem constants (hardcoded per harness contract)
B, L, D = 4, 2048, 384
HEADS = 8
HD = 48  # head dim
H_LOC = 4  # heads per core
EXPAND = 2
EPS = 1e-5
NTOK = B * L  # tokens for batchnorm stats
N_CORES = 8
QT = 512  # q tile width
NQT = L // QT  # 4 q tiles
NKT = L // 128  # 16 k tiles
DC = D // 128  # 3 d-chunks
SCALE = 1.0 / math.sqrt(HD)

# qkvwT packed column layout: [Qpack 256 | Kpack 256 | V 192 | pre 384]
QCOL, KCOL, VCOL, PCOL = 0, 256, 512, 704
QKVW_COLS = 1088

LAST_EXEC_TIME_NS = None
LAST_RESULTS = None

_PROGRAM_CACHE = {}


def _build_program(split_waits=True, gelu_func=None):
    gelu = gelu_func or AF.Gelu_apprx_tanh
    nc = bass.Bass()

    # ---- I/O ----
    xT = nc.dram_tensor("xT", [D, NTOK], BF16, kind="ExternalInput")
    xres = nc.dram_tensor("xres", [L, D], F32, kind="ExternalInput")
    gamma = nc.dram_tensor("gamma", [128, DC], F32, kind="ExternalInput")
    beta = nc.dram_tensor("beta", [128, DC], F32, kind="ExternalInput")
    qkvwT = nc.dram_tensor("qkvwT", [D, QKVW_COLS], BF16, kind="ExternalInput")
    qkvb = nc.dram_tensor("qkvb", [128, 4], F32, kind="ExternalInput")
    preb = nc.dram_tensor("preb", [128, DC], F32, kind="ExternalInput")
    vbias = nc.dram_tensor("vbias", [128, 2], F32, kind="ExternalInput")
    outwT = nc.dram_tensor("outwT", [7, 128, D], BF16, kind="ExternalInput")
    outb = nc.dram_tensor("outb", [1, D], BF16, kind="ExternalInput")
    masks = nc.dram_tensor("masks", [2, 128, 1024], BF16, kind="ExternalInput")
    ones_row = nc.dram_tensor("ones_row", [1, L], BF16, kind="ExternalInput")
    out = nc.dram_tensor("out", [L, D], F32, kind="ExternalOutput")

    with tile.TileContext(nc) as tc:
        with (
            tc.tile_pool(name="const", bufs=1) as const,
            tc.tile_pool(name="work", bufs=1) as work,
            tc.tile_pool(name="ptp", bufs=3) as ptp,
            tc.tile_pool(name="scr", bufs=2) as scr,
            tc.tile_pool(name="outp", bufs=3) as outpool,
            tc.tile_pool(name="nrm", bufs=1) as nrm,
            tc.tile_pool(name="psA", bufs=2, space="PSUM") as psA,
            tc.tile_pool(name="psB", bufs=2, space="PSUM") as psB,
            tc.tile_pool(name="psO", bufs=2, space="PSUM") as psO,
            tc.tile_pool(name="dram", bufs=1, space="DRAM") as dram,
        ):
            # ---- load inputs to SBUF ----
            xT_sb = []
            for c in range(DC):
                t = const.tile([128, NTOK], BF16, name=f"xT_sb{c}", tag=f"xT_sb{c}")
                for i in range(4):
                    w = NTOK // 4
                    nc.sync.dma_start(
                        out=t[:, w * i : w * (i + 1)],
                        in_=xT[128 * c : 128 * (c + 1), w * i : w * (i + 1)],
                    )
                xT_sb.append(t)

            w_sb = []
            for c in range(DC):
                t = const.tile([128, QKVW_COLS], BF16, name=f"w{c}", tag=f"w{c}")
                nc.sync.dma_start(out=t, in_=qkvwT[128 * c : 128 * (c + 1), :])
                w_sb.append(t)
            outw_sb = []
            for i in range(7):
                t = const.tile([128, D], BF16, name=f"outw{i}", tag=f"outw{i}")
                nc.sync.dma_start(out=t, in_=outwT[i])
                outw_sb.append(t)
            outb_sb = const.tile([1, D], BF16, name="outb_sb", tag="outb_sb")
            nc.sync.dma_start(out=outb_sb, in_=outb[:, :])
            masks_sb = []
            for i in range(2):
                t = const.tile([128, 1024], BF16, name=f"mask{i}", tag=f"mask{i}")
                nc.sync.dma_start(out=t, in_=masks[i])
                masks_sb.append(t)
            ones_sb = const.tile([1, L], BF16, name="ones_sb", tag="ones_sb")
            nc.sync.dma_start(out=ones_sb, in_=ones_row[:, :])
            ones33_sb = const.tile([33, 64], BF16, name="ones33_sb", tag="ones33_sb")
            nc.vector.memset(ones33_sb, 1.0)  # K=1 broadcast lhsT at base 0 and 32
            gamma_sb = const.tile([128, DC], F32, name="gamma_sb", tag="gamma_sb")
            nc.sync.dma_start(out=gamma_sb, in_=gamma[:, :])
            beta_sb = const.tile([128, DC], F32, name="beta_sb", tag="beta_sb")
            nc.sync.dma_start(out=beta_sb, in_=beta[:, :])
            qkvb_sb = const.tile([128, 4], F32, name="qkvb_sb", tag="qkvb_sb")
            nc.sync.dma_start(out=qkvb_sb, in_=qkvb[:, :])
            preb_sb = const.tile([128, DC], F32, name="preb_sb", tag="preb_sb")
            nc.sync.dma_start(out=preb_sb, in_=preb[:, :])
            vbias_sb = const.tile([128, 2], F32, name="vbias_sb", tag="vbias_sb")
            nc.sync.dma_start(out=vbias_sb, in_=vbias[:, :])

            # ---- batchnorm statistics (replicated over full x) ----
            # per d-chunk independent stats so hT_c unblocks as soon as its
            # chunk is reduced; chunk 1 runs on ScalarE to parallelize with DVE
            NCH = NTOK // 512
            eps_sb = work.tile([128, 1], F32, name="eps_sb", tag="eps_sb")
            nc.vector.memset(eps_sb, EPS)
            eps0_sb = work.tile([128, 1], F32, name="eps0_sb", tag="eps0_sb")
            nc.vector.memset(eps0_sb, 0.0)
            # pre-warm the ln/exp activation table set during the input DMAs
            warm = work.tile([128, 1], F32, name="warm", tag="warm")
            nc.vector.memset(warm, 1.0)
            nc.scalar.activation(out=warm, in_=warm, func=AF.Ln, bias=eps0_sb, scale=1.0)
            nc.scalar.activation(out=warm, in_=warm, func=AF.Exp, scale=1.0)
            s_sb = work.tile([128, DC], F32, name="s_sb", tag="s_sb")
            bb_sb = work.tile([128, DC], F32, name="bb_sb", tag="bb_sb")
            for c in range(DC):
                mv_c = work.tile([128, 2], F32, name=f"mv{c}", tag=f"mv{c}")
                if c == 1:
                    acc = work.tile([128, 2], F32, name=f"acc{c}", tag=f"acc{c}")
                    nc.scalar.activation(out=xT_sb[c], in_=xT_sb[c], func=AF.Copy,
                                         scale=1.0, accum_out=acc[:, 0:1])
                    sc2 = scr.tile([128, NTOK], BF16, name="actscr", tag="actscr",
                                   bufs=1)
                    nc.scalar.activation(out=sc2, in_=xT_sb[c], func=AF.Square,
                                         scale=1.0, accum_out=acc[:, 1:2])
                    # mean = sum/N ; var = sumsq/N - mean^2
                    nc.scalar.activation(out=mv_c[:, 0:1], in_=acc[:, 0:1],
                                         func=AF.Copy, scale=1.0 / NTOK)
                    ex2 = work.tile([128, 1], F32, name=f"ex2{c}", tag=f"ex2{c}")
                    nc.scalar.activation(out=ex2, in_=acc[:, 1:2],
                                         func=AF.Copy, scale=1.0 / NTOK)
                    nc.vector.scalar_tensor_tensor(
                        out=mv_c[:, 1:2], in0=mv_c[:, 0:1], scalar=-1.0,
                        in1=mv_c[:, 0:1], op0=ALU.mult, op1=ALU.mult)
                    nc.vector.tensor_add(out=mv_c[:, 1:2], in0=mv_c[:, 1:2], in1=ex2)
                else:
                    st6 = work.tile([128, NCH, 6], F32, name=f"st6_{c}", tag=f"st6_{c}")
                    for i in range(NCH):
                        nc.vector.bn_stats(
                            out=st6[:, i, :], in_=xT_sb[c][:, 512 * i : 512 * (i + 1)]
                        )
                    nc.vector.bn_aggr(out=mv_c, in_=st6)
                # s = rstd*gamma (rstd = exp(-0.5*ln(var+eps))); bb = beta - mean*s
                lnv = work.tile([128, 1], F32, name=f"lnv{c}", tag=f"lnv{c}")
                nc.scalar.activation(out=lnv, in_=mv_c[:, 1:2], func=AF.Ln,
                                     bias=eps_sb, scale=1.0)
                rstd = work.tile([128, 1], F32, name=f"rstd{c}", tag=f"rstd{c}")
                nc.scalar.activation(out=rstd, in_=lnv, func=AF.Exp, scale=-0.5)
                nc.vector.tensor_mul(out=s_sb[:, c : c + 1], in0=rstd,
                                     in1=gamma_sb[:, c : c + 1])
                nc.vector.scalar_tensor_tensor(
                    out=bb_sb[:, c : c + 1], in0=mv_c[:, 0:1], scalar=-1.0,
                    in1=s_sb[:, c : c + 1], op0=ALU.mult, op1=ALU.mult)
                nc.vector.tensor_add(out=bb_sb[:, c : c + 1],
                                     in0=bb_sb[:, c : c + 1], in1=beta_sb[:, c : c + 1])

            # ---- normalize own batch -> hT (bf16) ----
            # normalize in place over the own-batch columns of xT
            hT_sb = []
            for c in range(DC):
                t = xT_sb[c][:, 0:L]
                nc.vector.tensor_scalar(
                    out=t, in0=t, scalar1=s_sb[:, c : c + 1],
                    scalar2=bb_sb[:, c : c + 1], op0=ALU.mult, op1=ALU.add,
                )
                hT_sb.append(t)

            # ---- QKV projections ----
            # QT/KT: transposed [outch, tok]; heads packed 2-per-tile (48+48+32pad)
            QT_sb = [work.tile([128, L], BF16, name=f"QTt{m}", tag=f"QTt{m}") for m in range(2)]
            KT_sb = [work.tile([128, L], BF16, name=f"KTt{m}", tag=f"KTt{m}") for m in range(2)]
            for dst, col0, bcol0 in ((QT_sb, QCOL, 0), (KT_sb, KCOL, 2)):
                for m in range(2):
                    for tt in range(NQT):
                        ps = psB.tile([128, QT], F32, name="ps_qk", tag="psB")
                        for c in range(DC):
                            nc.tensor.matmul(
                                out=ps,
                                lhsT=w_sb[c][:, col0 + 128 * m : col0 + 128 * (m + 1)],
                                rhs=hT_sb[c][:, QT * tt : QT * (tt + 1)],
                                start=(c == 0), stop=(c == DC - 1),
                            )
                        nc.vector.tensor_scalar(
                            out=dst[m][:, QT * tt : QT * (tt + 1)], in0=ps,
                            scalar1=qkvb_sb[:, bcol0 + m : bcol0 + m + 1], scalar2=None,
                            op0=ALU.add,
                        )
            # V natural [tok, 4 heads * 49] with ones column per head
            V_sb = []
            for t16 in range(16):
                vt = work.tile([128, H_LOC * 65], BF16, name=f"V{t16}", tag=f"V{t16}")
                nc.vector.memset(vt, 1.0)
                V_sb.append(vt)
            for t16 in range(16):
                ps = psB.tile([128, H_LOC * HD], F32, name="ps_v", tag="psB")
                for c in range(DC):
                    nc.tensor.matmul(
                        out=ps,
                        lhsT=hT_sb[c][:, 128 * t16 : 128 * (t16 + 1)],
                        rhs=w_sb[c][:, VCOL : VCOL + H_LOC * HD],
                        start=(c == 0), stop=(c == DC - 1),
                    )
                dstv = V_sb[t16].rearrange("p (h d) -> p h d", h=H_LOC)[:, :, 0:HD]
                nc.vector.tensor_copy(
                    out=dstv, in_=ps.rearrange("p (h d) -> p h d", h=H_LOC)
                )

            # ---- pre branch: gelu(tanh approx) ----
            G_sb = [work.tile([128, L], BF16, name=f"G{m}", tag=f"G{m}") for m in range(DC)]
            for m in range(DC):
                for tp in range(2):
                    ps = psA.tile([128, 1024], F32, name="ps_pre", tag="psA")
                    for half in range(2):
                        tt = 2 * tp + half
                        for c in range(DC):
                            nc.tensor.matmul(
                                out=ps[:, QT * half : QT * (half + 1)],
                                lhsT=w_sb[c][:, PCOL + 128 * m : PCOL + 128 * (m + 1)],
                                rhs=hT_sb[c][:, QT * tt : QT * (tt + 1)],
                                start=(c == 0), stop=(c == DC - 1),
                            )
                    nc.scalar.activation(
                        out=G_sb[m][:, 1024 * tp : 1024 * (tp + 1)], in_=ps,
                        func=gelu, bias=preb_sb[:, m : m + 1], scale=1.0,
                    )

            # ---- attention (4 local heads, causal) ----
            catO = [None] * 4
            OT_sb = [work.tile([128, L], BF16, name=f"OTt{m}", tag=f"OTt{m}") for m in range(2)]
            for m in range(2):
                nc.vector.memset(OT_sb[m], 0.0)
            for m in range(2):
                denom = nrm.tile([64, L], F32, name="denom", tag="denom")
                nc.vector.memset(denom, 0.0)
                for hh in range(2):
                    h, ko = 2 * m + hh, 64 * hh
                    for j in range(NQT):
                        ext = 4 * (j + 1)  # k tiles for this q tile
                        ot = psO.tile([65, QT], F32, name="ps_ot", tag="psO")
                        for p in range(ext // 2):
                            st = psA.tile([128, 1024], F32, name="ps_st", tag="psA")
                            for half in range(2):
                                t = 2 * p + half
                                nc.tensor.matmul(
                                    out=st[:, QT * half : QT * (half + 1)],
                                    lhsT=KT_sb[m][ko : ko + HD, 128 * t : 128 * (t + 1)],
                                    rhs=QT_sb[m][ko : ko + HD, QT * j : QT * (j + 1)],
                                    start=True, stop=True,
                                )
                            pt = ptp.tile([128, 1024], BF16, name="pt", tag="pt")
                            nc.scalar.activation(out=pt, in_=st, func=AF.Exp, scale=SCALE)
                            if p == 2 * j:
                                nc.vector.tensor_mul(out=pt, in0=pt, in1=masks_sb[0])
                            elif p == 2 * j + 1:
                                nc.vector.tensor_mul(out=pt, in0=pt, in1=masks_sb[1])
                            for half in range(2):
                                t = 2 * p + half
                                nc.tensor.matmul(
                                    out=ot,
                                    lhsT=V_sb[t][:, 65 * h : 65 * (h + 1)],
                                    rhs=pt[:, QT * half : QT * (half + 1)],
                                    start=(p == 0 and half == 0),
                                    stop=(p == ext // 2 - 1 and half == 1),
                                )
                        # softmax denominator (row 64) copied off on ScalarE;
                        # unnormalized o copied on DVE
                        nc.scalar.activation(
                            out=denom[32 * hh : 32 * hh + 1, QT * j : QT * (j + 1)],
                            in_=ot[64:65, :], func=AF.Copy, scale=1.0,
                        )
                        nc.vector.tensor_copy(
                            out=OT_sb[m][ko : ko + HD, QT * j : QT * (j + 1)],
                            in_=ot[0:HD, :],
                        )
                # batched reciprocal 1/d = exp(-ln d) on ScalarE (same table
                # set as the attention exp); bf16 out for the PE broadcast
                nc.scalar.activation(out=denom, in_=denom, func=AF.Ln,
                                     bias=eps0_sb[0:64, :], scale=1.0)
                recipb = nrm.tile([64, L], BF16, name="recipb", tag="recipb", bufs=1)
                nc.scalar.activation(out=recipb, in_=denom, func=AF.Exp, scale=-1.0)
                # broadcast each 1/denom row over 64 partitions via K=1 matmuls
                # into PSUM, then one multiply per q-tile (one PSUM operand ok)
                for j in range(NQT):
                    bc = psB.tile([128, QT], F32, name="ps_bc", tag="psB")
                    nc.tensor.matmul(
                        out=bc[0:64, :], lhsT=ones33_sb[0:1, :],
                        rhs=recipb[0:1, QT * j : QT * (j + 1)],
                        start=True, stop=True,
                    )
                    nc.tensor.matmul(
                        out=bc[64:128, :], lhsT=ones33_sb[32:33, :],
                        rhs=recipb[32:33, QT * j : QT * (j + 1)],
                        start=True, stop=True,
                    )
                    nc.vector.tensor_mul(
                        out=OT_sb[m][:, QT * j : QT * (j + 1)],
                        in0=OT_sb[m][:, QT * j : QT * (j + 1)], in1=bc,
                    )
                # V bias (usually zero): per-partition add
                nc.vector.tensor_scalar(
                    out=OT_sb[m], in0=OT_sb[m], scalar1=vbias_sb[:, m : m + 1],
                    scalar2=None, op0=ALU.add,
                )
                # exchange this head-pair within the batch pair (AllGather);
                # m=0's collective overlaps m=1's attention
                oswap_in = dram.tile([128, L], BF16, name=f"oswap_in{m}",
                                     tag=f"oswap_in{m}")
                oswap_out = dram.tile([256, L], BF16, name=f"oswap_out{m}",
                                      tag=f"oswap_out{m}")
                nc.sync.dma_start(out=oswap_in[:, :], in_=OT_sb[m])
                nc.gpsimd.collective_compute(
                    "AllGather", ALU.bypass,
                    replica_groups=[[0, 1], [2, 3], [4, 5], [6, 7]],
                    ins=[oswap_in.opt()], outs=[oswap_out.opt()],
                )
                # AG_m blocks: [my pair-m heads | partner pair-m heads] ->
                # global head order: catO[m] = blk0, catO[m+2] = blk1
                t = work.tile([128, L], BF16, name=f"catO{m}", tag=f"catO{m}")
                nc.gpsimd.dma_start(out=t, in_=oswap_out[0:128, :])
                catO[m] = t
                t2 = work.tile([128, L], BF16, name=f"catO{m + 2}", tag=f"catO{m + 2}")
                nc.gpsimd.dma_start(out=t2, in_=oswap_out[128:256, :])
                catO[m + 2] = t2

            # ---- out projection + residual ----
            # two-phase accumulation: phase A = gelu chunks + bias + AG0 blocks
            # (copied to SBUF on ScalarE while AG1 is in flight); phase B = AG1
            # blocks only, so the post-collective tail is 2 matmuls + adds
            xr_sb = []
            for t16 in range(16):
                xr = outpool.tile([128, D], F32, name="xr", tag=f"xr{t16 % 4}", bufs=1)
                nc.sync.dma_start(out=xr, in_=xres[128 * t16 : 128 * (t16 + 1), :])
                xr_sb.append(xr)
            cat_tiles = G_sb + catO  # 7 chunks matching outwT order
            phaseA = [0, 1, 2, -1, 3, 5]  # -1 = bias row; 3,5 = AG0 blocks
            phaseB = [4, 6]  # AG1 blocks
            out_acc = []
            for t16 in range(16):
                po = psB.tile([128, D], F32, name="ps_out", tag="psB")
                for ci, i in enumerate(phaseA):
                    if i < 0:
                        nc.tensor.matmul(
                            out=po,
                            lhsT=ones_sb[:, 128 * t16 : 128 * (t16 + 1)],
                            rhs=outb_sb,
                            start=False, stop=False,
                        )
                    else:
                        nc.tensor.matmul(
                            out=po,
                            lhsT=cat_tiles[i][:, 128 * t16 : 128 * (t16 + 1)],
                            rhs=outw_sb[i],
                            start=(ci == 0), stop=(ci == len(phaseA) - 1),
                        )
                oa = outpool.tile([128, D], F32, name="oa", tag=f"oa{t16 % 8}",
                                  bufs=2)
                nc.vector.scalar_tensor_tensor(
                    out=oa, in0=po, scalar=1.0, in1=xr_sb[t16],
                    op0=ALU.mult, op1=ALU.add,
                )
                out_acc.append(oa)
            for t16 in range(16):
                po = psB.tile([128, D], F32, name="ps_out", tag="psB")
                for ci, i in enumerate(phaseB):
                    nc.tensor.matmul(
                        out=po,
                        lhsT=cat_tiles[i][:, 128 * t16 : 128 * (t16 + 1)],
                        rhs=outw_sb[i],
                        start=(ci == 0), stop=(ci == len(phaseB) - 1),
                    )
                ro = outpool.tile([128, D], F32, name="ro", tag="ro")
                nc.vector.scalar_tensor_tensor(
                    out=ro, in0=po, scalar=1.0, in1=out_acc[t16],
                    op0=ALU.mult, op1=ALU.add,
                )
                nc.gpsimd.dma_start(out=out[128 * t16 : 128 * (t16 + 1), :], in_=ro)

    if split_waits:
        _split_multi_waits(nc)
    return nc


def _split_multi_waits(nc):
    """This toolchain's walrus encodes at most one sync-wait per instruction;
    hoist extra waits into standalone EventSemaphore instructions on the same
    engine immediately before the original instruction."""
    for bb in nc.main_func.blocks:
        insts = list(bb.instructions)
        if not any(
            ins.sync_info is not None and len(ins.sync_info.on_wait) > 1
            for ins in insts
        ):
            continue
        new = []
        for ins in insts:
            si = ins.sync_info
            if si is not None and len(si.on_wait) > 1:
                waits = list(si.on_wait)
                for k, w in enumerate(waits[:-1]):
                    es = mybir.InstEventSemaphore(name=f"{ins.name}-w{k}", ins=[], outs=[])
                    es.engine = ins.engine
                    es.sync_info = _bass_rust.SyncInfo(on_wait=[w], on_update=[])
                    new.append(es)
                ins.sync_info = _bass_rust.SyncInfo(
                    on_wait=[waits[-1]], on_update=list(si.on_update)
                )
            new.append(ins)
        bb.instructions = new


def _prep_core_inputs(x, norm_gamma, norm_beta, qkv_w, qkv_b, out_w, out_b, core):
    bf16 = ml_dtypes.bfloat16
    b, s = core // 2, core % 2
    heads = [4 * s + i for i in range(H_LOC)]

    # xT: own batch first, remaining batches after (stats are order-invariant)
    other = [bb for bb in range(B) if bb != b]
    xt_cols = [x[b].T] + [x[bb].T for bb in other]
    xT = np.ascontiguousarray(np.concatenate(xt_cols, axis=1)).astype(bf16)

    xres = np.ascontiguousarray(x[b]).astype(np.float32)

    gamma = np.ascontiguousarray(norm_gamma.reshape(DC, 128).T).astype(np.float32)
    beta = np.ascontiguousarray(norm_beta.reshape(DC, 128).T).astype(np.float32)

    wq, wk, wv, wpre = (qkv_w[i * D : (i + 1) * D] for i in range(4))
    bq, bk, bv, bpre = (qkv_b[i * D : (i + 1) * D] for i in range(4))

    qkvwT = np.zeros((D, QKVW_COLS), np.float32)
    qkvb_arr = np.zeros((128, 4), np.float32)
    for m in range(2):
        for hh in range(2):
            hglob = heads[2 * m + hh]
            rows = slice(HD * hglob, HD * (hglob + 1))
            o0 = 64 * hh
            qkvwT[:, QCOL + 128 * m + o0 : QCOL + 128 * m + o0 + HD] = wq[rows].T
            qkvwT[:, KCOL + 128 * m + o0 : KCOL + 128 * m + o0 + HD] = wk[rows].T
            qkvb_arr[o0 : o0 + HD, m] = bq[rows]
            qkvb_arr[o0 : o0 + HD, 2 + m] = bk[rows]
    qkvwT[:, VCOL : VCOL + H_LOC * HD] = wv[HD * heads[0] : HD * (heads[-1] + 1)].T
    qkvwT[:, PCOL : PCOL + D] = wpre.T
    qkvwT = qkvwT.astype(bf16)

    preb = np.ascontiguousarray(bpre.reshape(DC, 128).T).astype(np.float32)
    vb = np.zeros((128, 2), np.float32)
    for m in range(2):
        for hh in range(2):
            hglob = heads[2 * m + hh]
            vb[64 * hh : 64 * hh + HD, m] = bv[HD * hglob : HD * (hglob + 1)]

    # out projection: out_w [D, 768]; rows 0:384 gelu(pre) dims, 384:768 o dims
    owT = out_w.T.astype(np.float32)  # [768, D]
    outwT = np.zeros((7, 128, D), np.float32)
    for m in range(DC):
        outwT[m] = owT[128 * m : 128 * (m + 1)]
    for p2 in range(4):  # o chunks, global head order, heads at row offsets 0/64
        outwT[DC + p2, 0:HD, :] = owT[D + HD * 2 * p2 : D + HD * (2 * p2 + 1)]
        outwT[DC + p2, 64 : 64 + HD, :] = owT[D + HD * (2 * p2 + 1) : D + HD * (2 * p2 + 2)]
    outwT = outwT.astype(bf16)
    outb_arr = out_b.reshape(1, D).astype(bf16)

    # causal masks for diagonal k-tile pairs: mask_dd[kl, ql] = ql >= kl + 128*dd
    ql = np.arange(QT)[None, :]
    kl = np.arange(128)[:, None]
    m4 = [(ql >= kl + 128 * dd).astype(np.float32) for dd in range(4)]
    masks = np.zeros((2, 128, 1024), np.float32)
    masks[0, :, 0:QT] = m4[0]
    masks[0, :, QT:1024] = m4[1]
    masks[1, :, 0:QT] = m4[2]
    masks[1, :, QT:1024] = m4[3]
    masks = masks.astype(bf16)

    ones_arr = np.ones((1, L), bf16)

    return {
        "xT": xT, "xres": xres, "gamma": gamma, "beta": beta,
        "qkvwT": qkvwT, "qkvb": qkvb_arr, "preb": preb, "vbias": vb,
        "outwT": outwT, "outb": outb_arr, "masks": masks, "ones_row": ones_arr,
    }


def _install_ntff_shim():
    """Provide antenv.axon_hooks (absent in this image) so bass_utils'
    trace path can reach the axon NTFF profiler via ctypes."""
    try:
        import sys, types
        import antenv
        if "antenv.axon_hooks" not in sys.modules:
            from trn_agent_boot.trn_boot import _ntff_profile_via_ctypes
            hook = _ntff_profile_via_ctypes("/opt/axon/libaxon_pjrt.so")
            mod = types.ModuleType("antenv.axon_hooks")
            mod._hook = hook
            mod.set_axon_ntff_profile_hook = lambda h: setattr(mod, "_hook", h)
            mod.get_axon_ntff_profile_hook = lambda: mod._hook
            sys.modules["antenv.axon_hooks"] = mod
            antenv.axon_hooks = mod
        import concourse.bass_utils as _bu
        _bu.upload_artifacts = lambda d: "local"
        return True
    except Exception as e:
        print(f"ntff shim unavailable: {e!r}")
        return False


def kernel(x, norm_gamma, norm_beta, qkv_w, qkv_b, out_w, out_b):
    global LAST_EXEC_TIME_NS, LAST_RESULTS
    x = np.asarray(x, np.float32)
    norm_gamma = np.asarray(norm_gamma, np.float32)
    norm_beta = np.asarray(norm_beta, np.float32)
    qkv_w = np.asarray(qkv_w, np.float32)
    qkv_b = np.asarray(qkv_b, np.float32)
    out_w = np.asarray(out_w, np.float32)
    out_b = np.asarray(out_b, np.float32)

    if "nc" not in _PROGRAM_CACHE:
        _PROGRAM_CACHE["nc"] = _build_program()
    nc = _PROGRAM_CACHE["nc"]

    in_maps = [
        _prep_core_inputs(x, norm_gamma, norm_beta, qkv_w, qkv_b, out_w, out_b, c)
        for c in range(N_CORES)
    ]
    trace = os.environ.get("KERNEL_TRACE", "0") == "1"
    if trace:
        trace = _install_ntff_shim()
    res = run_bass_kernel_spmd(
        nc, in_maps, list(range(N_CORES)), trace=trace,
        trace_cores=list(range(N_CORES)) if trace else None,
    )
    LAST_EXEC_TIME_NS = res.exec_time_ns
    LAST_RESULTS = res
    out = np.empty((B, L, D), np.float32)
    for b in range(B):
        out[b] = res.results[2 * b]["out"]
    return out


# revision 31
# speedup vs baseline: 1.2505x; 1.0171x over previous
"""Trainium2 Bass kernel for nn_Block (BatchNorm -> QKV -> causal MHA + gelu gate -> out proj + residual).

Contract: kernel(**inputs) takes FULL unsharded inputs (np arrays, keys as in
setup_inputs()) and returns the FULL output (4, 2048, 384) float32.

Sharding: 8 cores; core c handles batch b=c//2 and head-half s=c%2 (4 of 8 heads).
Both cores of a batch pair compute the full residual output for their batch
(symmetric SPMD program); o-halves are exchanged with a 2-rank AllGather.
BatchNorm statistics are computed replicated on every core from a
column-permuted transposed copy of the full x (own batch's 2048 columns first,
so the program is identical across cores).
"""

import math
import os

import numpy as np
import ml_dtypes

import bass_rust as _bass_rust
import concourse.bass as bass
import concourse.tile as tile
import concourse.mybir as mybir
from concourse.bass_utils import run_bass_kernel_spmd

BF16 = mybir.dt.bfloat16
F32 = mybir.dt.float32
AF = mybir.ActivationFunctionType
ALU = mybir.AluOpType

# Problem constants (hardcoded per harness contract)
B, L, D = 4, 2048, 384
HEADS = 8
HD = 48  # head dim
H_LOC = 4  # heads per core
EXPAND = 2
EPS = 1e-5
NTOK = B * L  # tokens for batchnorm stats
N_CORES = 8
QT = 512  # q tile width
NQT = L // QT  # 4 q tiles
NKT = L // 128  # 16 k tiles
DC = D // 128  # 3 d-chunks
DOUT = 192  # out channels per core (pair splits D)
SCALE = 1.0 / math.sqrt(HD)

# qkvwT packed column layout: [Qpack 256 | Kpack 256 | V 192 | pre 384]
QCOL, KCOL, VCOL, PCOL = 0, 256, 512, 704
QKVW_COLS = 1088

LAST_EXEC_TIME_NS = None
LAST_RESULTS = None

_PROGRAM_CACHE = {}


def _build_program(split_waits=True, gelu_func=None):
    gelu = gelu_func or AF.Gelu_apprx_tanh
    nc = bass.Bass()

    # ---- I/O ----
    xT = nc.dram_tensor("xT", [D, NTOK], BF16, kind="ExternalInput")
    xres = nc.dram_tensor("xres", [L, DOUT], F32, kind="ExternalInput")
    gamma = nc.dram_tensor("gamma", [128, DC], F32, kind="ExternalInput")
    beta = nc.dram_tensor("beta", [128, DC], F32, kind="ExternalInput")
    qkvwT = nc.dram_tensor("qkvwT", [D, QKVW_COLS], BF16, kind="ExternalInput")
    qkvb = nc.dram_tensor("qkvb", [128, 4], F32, kind="ExternalInput")
    preb = nc.dram_tensor("preb", [128, DC], F32, kind="ExternalInput")
    vbias = nc.dram_tensor("vbias", [128, 2], F32, kind="ExternalInput")
    outwT = nc.dram_tensor("outwT", [7, 128, DOUT], BF16, kind="ExternalInput")
    outb = nc.dram_tensor("outb", [1, DOUT], BF16, kind="ExternalInput")
    masks = nc.dram_tensor("masks", [2, 128, 1024], BF16, kind="ExternalInput")
    ones_row = nc.dram_tensor("ones_row", [1, L], BF16, kind="ExternalInput")
    out = nc.dram_tensor("out", [L, DOUT], F32, kind="ExternalOutput")

    with tile.TileContext(nc) as tc:
        with (
            tc.tile_pool(name="const", bufs=1) as const,
            tc.tile_pool(name="work", bufs=1) as work,
            tc.tile_pool(name="ptp", bufs=3) as ptp,
            tc.tile_pool(name="scr", bufs=2) as scr,
            tc.tile_pool(name="outp", bufs=3) as outpool,
            tc.tile_pool(name="nrm", bufs=1) as nrm,
            tc.tile_pool(name="psA", bufs=2, space="PSUM") as psA,
            tc.tile_pool(name="psB", bufs=2, space="PSUM") as psB,
            tc.tile_pool(name="psO", bufs=2, space="PSUM") as psO,
            tc.tile_pool(name="dram", bufs=1, space="DRAM") as dram,
        ):
            # ---- load inputs to SBUF ----
            xT_sb = []
            for c in range(DC):
                t = const.tile([128, NTOK], BF16, name=f"xT_sb{c}", tag=f"xT_sb{c}")
                for i in range(4):
                    w = NTOK // 4
                    nc.sync.dma_start(
                        out=t[:, w * i : w * (i + 1)],
                        in_=xT[128 * c : 128 * (c + 1), w * i : w * (i + 1)],
                    )
                xT_sb.append(t)

            w_sb = []
            for c in range(DC):
                t = const.tile([128, QKVW_COLS], BF16, name=f"w{c}", tag=f"w{c}")
                nc.sync.dma_start(out=t, in_=qkvwT[128 * c : 128 * (c + 1), :])
                w_sb.append(t)
            outw_sb = []
            for i in range(7):
                t = const.tile([128, DOUT], BF16, name=f"outw{i}", tag=f"outw{i}")
                nc.sync.dma_start(out=t, in_=outwT[i])
                outw_sb.append(t)
            outb_sb = const.tile([1, DOUT], BF16, name="outb_sb", tag="outb_sb")
            nc.sync.dma_start(out=outb_sb, in_=outb[:, :])
            masks_sb = []
            for i in range(2):
                t = const.tile([128, 1024], BF16, name=f"mask{i}", tag=f"mask{i}")
                nc.sync.dma_start(out=t, in_=masks[i])
                masks_sb.append(t)
            ones_sb = const.tile([1, L], BF16, name="ones_sb", tag="ones_sb")
            nc.sync.dma_start(out=ones_sb, in_=ones_row[:, :])
            ones33_sb = const.tile([33, 64], BF16, name="ones33_sb", tag="ones33_sb")
            nc.vector.memset(ones33_sb, 1.0)  # K=1 broadcast lhsT at base 0 and 32
            gamma_sb = const.tile([128, DC], F32, name="gamma_sb", tag="gamma_sb")
            nc.sync.dma_start(out=gamma_sb, in_=gamma[:, :])
            beta_sb = const.tile([128, DC], F32, name="beta_sb", tag="beta_sb")
            nc.sync.dma_start(out=beta_sb, in_=beta[:, :])
            qkvb_sb = const.tile([128, 4], F32, name="qkvb_sb", tag="qkvb_sb")
            nc.sync.dma_start(out=qkvb_sb, in_=qkvb[:, :])
            preb_sb = const.tile([128, DC], F32, name="preb_sb", tag="preb_sb")
            nc.sync.dma_start(out=preb_sb, in_=preb[:, :])
            vbias_sb = const.tile([128, 2], F32, name="vbias_sb", tag="vbias_sb")
            nc.sync.dma_start(out=vbias_sb, in_=vbias[:, :])

            # ---- batchnorm statistics (replicated over full x) ----
            # per d-chunk independent stats so hT_c unblocks as soon as its
            # chunk is reduced; chunk 1 runs on ScalarE to parallelize with DVE
            NCH = NTOK // 512
            eps_sb = work.tile([128, 1], F32, name="eps_sb", tag="eps_sb")
            nc.vector.memset(eps_sb, EPS)
            eps0_sb = work.tile([128, 1], F32, name="eps0_sb", tag="eps0_sb")
            nc.vector.memset(eps0_sb, 0.0)
            # pre-warm the ln/exp activation table set during the input DMAs
            warm = work.tile([128, 1], F32, name="warm", tag="warm")
            nc.vector.memset(warm, 1.0)
            nc.scalar.activation(out=warm, in_=warm, func=AF.Ln, bias=eps0_sb, scale=1.0)
            nc.scalar.activation(out=warm, in_=warm, func=AF.Exp, scale=1.0)
            s_sb = work.tile([128, DC], F32, name="s_sb", tag="s_sb")
            bb_sb = work.tile([128, DC], F32, name="bb_sb", tag="bb_sb")
            hT_sb = []  # filled per chunk (in-place over xT own-batch columns)
            for c in range(DC):
                mv_c = work.tile([128, 2], F32, name=f"mv{c}", tag=f"mv{c}")
                if c == 1:
                    acc = work.tile([128, 2], F32, name=f"acc{c}", tag=f"acc{c}")
                    nc.scalar.activation(out=xT_sb[c], in_=xT_sb[c], func=AF.Copy,
                                         scale=1.0, accum_out=acc[:, 0:1])
                    sc2 = scr.tile([128, NTOK], BF16, name="actscr", tag="actscr",
                                   bufs=1)
                    nc.scalar.activation(out=sc2, in_=xT_sb[c], func=AF.Square,
                                         scale=1.0, accum_out=acc[:, 1:2])
                    # mean = sum/N ; var = sumsq/N - mean^2
                    nc.scalar.activation(out=mv_c[:, 0:1], in_=acc[:, 0:1],
                                         func=AF.Copy, scale=1.0 / NTOK)
                    ex2 = work.tile([128, 1], F32, name=f"ex2{c}", tag=f"ex2{c}")
                    nc.scalar.activation(out=ex2, in_=acc[:, 1:2],
                                         func=AF.Copy, scale=1.0 / NTOK)
                    nc.vector.scalar_tensor_tensor(
                        out=mv_c[:, 1:2], in0=mv_c[:, 0:1], scalar=-1.0,
                        in1=mv_c[:, 0:1], op0=ALU.mult, op1=ALU.mult)
                    nc.vector.tensor_add(out=mv_c[:, 1:2], in0=mv_c[:, 1:2], in1=ex2)
                else:
                    st6 = work.tile([128, NCH, 6], F32, name=f"st6_{c}", tag=f"st6_{c}")
                    for i in range(NCH):
                        nc.vector.bn_stats(
                            out=st6[:, i, :], in_=xT_sb[c][:, 512 * i : 512 * (i + 1)]
                        )
                    nc.vector.bn_aggr(out=mv_c, in_=st6)
                # s = rstd*gamma (rstd = exp(-0.5*ln(var+eps))); bb = beta - mean*s
                lnv = work.tile([128, 1], F32, name=f"lnv{c}", tag=f"lnv{c}")
                nc.scalar.activation(out=lnv, in_=mv_c[:, 1:2], func=AF.Ln,
                                     bias=eps_sb, scale=1.0)
                rstd = work.tile([128, 1], F32, name=f"rstd{c}", tag=f"rstd{c}")
                nc.scalar.activation(out=rstd, in_=lnv, func=AF.Exp, scale=-0.5)
                nc.vector.tensor_mul(out=s_sb[:, c : c + 1], in0=rstd,
                                     in1=gamma_sb[:, c : c + 1])
                nc.vector.scalar_tensor_tensor(
                    out=bb_sb[:, c : c + 1], in0=mv_c[:, 0:1], scalar=-1.0,
                    in1=s_sb[:, c : c + 1], op0=ALU.mult, op1=ALU.mult)
                nc.vector.tensor_add(out=bb_sb[:, c : c + 1],
                                     in0=bb_sb[:, c : c + 1], in1=beta_sb[:, c : c + 1])
                t = xT_sb[c][:, 0:L]
                nc.vector.tensor_scalar(
                    out=t, in0=t, scalar1=s_sb[:, c : c + 1],
                    scalar2=bb_sb[:, c : c + 1], op0=ALU.mult, op1=ALU.add,
                )
                hT_sb.append(t)

            # ---- normalize own batch -> hT (bf16) ----


            # ---- QKV projections ----
            # QT/KT: transposed [outch, tok]; heads packed 2-per-tile (48+48+32pad)
            QT_sb = [work.tile([128, L], BF16, name=f"QTt{m}", tag=f"QTt{m}") for m in range(2)]
            KT_sb = [work.tile([128, L], BF16, name=f"KTt{m}", tag=f"KTt{m}") for m in range(2)]
            for dst, col0, bcol0 in ((QT_sb, QCOL, 0), (KT_sb, KCOL, 2)):
                for m in range(2):
                    for tt in range(NQT):
                        ps = psB.tile([128, QT], F32, name="ps_qk", tag="psB")
                        for c in range(DC):
                            nc.tensor.matmul(
                                out=ps,
                                lhsT=w_sb[c][:, col0 + 128 * m : col0 + 128 * (m + 1)],
                                rhs=hT_sb[c][:, QT * tt : QT * (tt + 1)],
                                start=(c == 0), stop=(c == DC - 1),
                            )
                        nc.vector.tensor_scalar(
                            out=dst[m][:, QT * tt : QT * (tt + 1)], in0=ps,
                            scalar1=qkvb_sb[:, bcol0 + m : bcol0 + m + 1], scalar2=None,
                            op0=ALU.add,
                        )
            # V natural [tok, 4 heads * 49] with ones column per head
            V_sb = []
            for t16 in range(16):
                vt = work.tile([128, H_LOC * 65], BF16, name=f"V{t16}", tag=f"V{t16}")
                nc.vector.memset(vt, 1.0)
                V_sb.append(vt)
            for t16 in range(16):
                ps = psB.tile([128, H_LOC * HD], F32, name="ps_v", tag="psB")
                for c in range(DC):
                    nc.tensor.matmul(
                        out=ps,
                        lhsT=hT_sb[c][:, 128 * t16 : 128 * (t16 + 1)],
                        rhs=w_sb[c][:, VCOL : VCOL + H_LOC * HD],
                        start=(c == 0), stop=(c == DC - 1),
                    )
                dstv = V_sb[t16].rearrange("p (h d) -> p h d", h=H_LOC)[:, :, 0:HD]
                nc.vector.tensor_copy(
                    out=dstv, in_=ps.rearrange("p (h d) -> p h d", h=H_LOC)
                )

            # ---- pre branch: gelu(tanh approx) ----
            G_sb = [work.tile([128, L], BF16, name=f"G{m}", tag=f"G{m}") for m in range(DC)]
            for m in range(DC):
                for tp in range(2):
                    ps = psA.tile([128, 1024], F32, name="ps_pre", tag="psA")
                    for half in range(2):
                        tt = 2 * tp + half
                        for c in range(DC):
                            nc.tensor.matmul(
                                out=ps[:, QT * half : QT * (half + 1)],
                                lhsT=w_sb[c][:, PCOL + 128 * m : PCOL + 128 * (m + 1)],
                                rhs=hT_sb[c][:, QT * tt : QT * (tt + 1)],
                                start=(c == 0), stop=(c == DC - 1),
                            )
                    nc.scalar.activation(
                        out=G_sb[m][:, 1024 * tp : 1024 * (tp + 1)], in_=ps,
                        func=gelu, bias=preb_sb[:, m : m + 1], scale=1.0,
                    )

            # ---- attention (4 local heads, causal) ----
            catO = [None] * 4
            OT_sb = [work.tile([128, L], BF16, name=f"OTt{m}", tag=f"OTt{m}") for m in range(2)]
            for m in range(2):
                nc.vector.memset(OT_sb[m], 0.0)
            for m in range(2):
                denom = nrm.tile([64, L], F32, name="denom", tag="denom")
                nc.vector.memset(denom, 0.0)
                for hh in range(2):
                    h, ko = 2 * m + hh, 64 * hh
                    for j in range(NQT):
                        ext = 4 * (j + 1)  # k tiles for this q tile
                        ot = psO.tile([65, QT], F32, name="ps_ot", tag="psO")
                        for p in range(ext // 2):
                            st = psA.tile([128, 1024], F32, name="ps_st", tag="psA")
                            for half in range(2):
                                t = 2 * p + half
                                nc.tensor.matmul(
                                    out=st[:, QT * half : QT * (half + 1)],
                                    lhsT=KT_sb[m][ko : ko + HD, 128 * t : 128 * (t + 1)],
                                    rhs=QT_sb[m][ko : ko + HD, QT * j : QT * (j + 1)],
                                    start=True, stop=True,
                                )
                            pt = ptp.tile([128, 1024], BF16, name="pt", tag="pt")
                            nc.scalar.activation(out=pt, in_=st, func=AF.Exp, scale=SCALE)
                            if p == 2 * j:
                                nc.vector.tensor_mul(out=pt, in0=pt, in1=masks_sb[0])
                            elif p == 2 * j + 1:
                                nc.vector.tensor_mul(out=pt, in0=pt, in1=masks_sb[1])
                            for half in range(2):
                                t = 2 * p + half
                                nc.tensor.matmul(
                                    out=ot,
                                    lhsT=V_sb[t][:, 65 * h : 65 * (h + 1)],
                                    rhs=pt[:, QT * half : QT * (half + 1)],
                                    start=(p == 0 and half == 0),
                                    stop=(p == ext // 2 - 1 and half == 1),
                                )
                        # softmax denominator (row 64) copied off on ScalarE;
                        # unnormalized o copied on DVE
                        nc.scalar.activation(
                            out=denom[32 * hh : 32 * hh + 1, QT * j : QT * (j + 1)],
                            in_=ot[64:65, :], func=AF.Copy, scale=1.0,
                        )
                        nc.vector.tensor_copy(
                            out=OT_sb[m][ko : ko + HD, QT * j : QT * (j + 1)],
                            in_=ot[0:HD, :],
                        )
                # batched reciprocal 1/d = exp(-ln d) on ScalarE (same table
                # set as the attention exp); bf16 out for the PE broadcast
                nc.scalar.activation(out=denom, in_=denom, func=AF.Ln,
                                     bias=eps0_sb[0:64, :], scale=1.0)
                recipb = nrm.tile([64, L], BF16, name="recipb", tag="recipb", bufs=1)
                nc.scalar.activation(out=recipb, in_=denom, func=AF.Exp, scale=-1.0)
                # broadcast each 1/denom row over 64 partitions via K=1 matmuls
                # into PSUM, then one multiply per q-tile (one PSUM operand ok)
                for j in range(NQT):
                    bc = psB.tile([128, QT], F32, name="ps_bc", tag="psB")
                    nc.tensor.matmul(
                        out=bc[0:64, :], lhsT=ones33_sb[0:1, :],
                        rhs=recipb[0:1, QT * j : QT * (j + 1)],
                        start=True, stop=True,
                    )
                    nc.tensor.matmul(
                        out=bc[64:128, :], lhsT=ones33_sb[32:33, :],
                        rhs=recipb[32:33, QT * j : QT * (j + 1)],
                        start=True, stop=True,
                    )
                    nc.vector.tensor_mul(
                        out=OT_sb[m][:, QT * j : QT * (j + 1)],
                        in0=OT_sb[m][:, QT * j : QT * (j + 1)], in1=bc,
                    )
                # V bias (usually zero): per-partition add
                nc.vector.tensor_scalar(
                    out=OT_sb[m], in0=OT_sb[m], scalar1=vbias_sb[:, m : m + 1],
                    scalar2=None, op0=ALU.add,
                )
                # exchange this head-pair within the batch pair (AllGather);
                # m=0's collective overlaps m=1's attention
                oswap_in = dram.tile([128, L], BF16, name=f"oswap_in{m}",
                                     tag=f"oswap_in{m}")
                oswap_out = dram.tile([256, L], BF16, name=f"oswap_out{m}",
                                      tag=f"oswap_out{m}")
                nc.sync.dma_start(out=oswap_in[:, :], in_=OT_sb[m])
                nc.gpsimd.collective_compute(
                    "AllGather", ALU.bypass,
                    replica_groups=[[0, 1], [2, 3], [4, 5], [6, 7]],
                    ins=[oswap_in.opt()], outs=[oswap_out.opt()],
                )
                # AG_m blocks: [my pair-m heads | partner pair-m heads] ->
                # global head order: catO[m] = blk0, catO[m+2] = blk1
                t = work.tile([128, L], BF16, name=f"catO{m}", tag=f"catO{m}")
                nc.gpsimd.dma_start(out=t, in_=oswap_out[0:128, :])
                catO[m] = t
                t2 = work.tile([128, L], BF16, name=f"catO{m + 2}", tag=f"catO{m + 2}")
                nc.gpsimd.dma_start(out=t2, in_=oswap_out[128:256, :])
                catO[m + 2] = t2

            # ---- out projection + residual ----
            # two-phase accumulation: phase A = gelu chunks + bias + AG0 blocks
            # (copied to SBUF on ScalarE while AG1 is in flight); phase B = AG1
            # blocks only, so the post-collective tail is 2 matmuls + adds
            xr_sb = []
            for t16 in range(16):
                xr = outpool.tile([128, DOUT], F32, name="xr", tag=f"xr{t16 % 4}", bufs=1)
                nc.sync.dma_start(out=xr, in_=xres[128 * t16 : 128 * (t16 + 1), :])
                xr_sb.append(xr)
            cat_tiles = G_sb + catO  # 7 chunks matching outwT order
            phaseA = [0, 1, 2, -1, 3, 5]  # -1 = bias row; 3,5 = AG0 blocks
            phaseB = [4, 6]  # AG1 blocks
            out_acc = []
            for t16 in range(16):
                po = psB.tile([128, DOUT], F32, name="ps_out", tag="psB")
                for ci, i in enumerate(phaseA):
                    if i < 0:
                        nc.tensor.matmul(
                            out=po,
                            lhsT=ones_sb[:, 128 * t16 : 128 * (t16 + 1)],
                            rhs=outb_sb,
                            start=False, stop=False,
                        )
                    else:
                        nc.tensor.matmul(
                            out=po,
                            lhsT=cat_tiles[i][:, 128 * t16 : 128 * (t16 + 1)],
                            rhs=outw_sb[i],
                            start=(ci == 0), stop=(ci == len(phaseA) - 1),
                        )
                oa = outpool.tile([128, DOUT], F32, name="oa", tag=f"oa{t16 % 8}",
                                  bufs=2)
                nc.vector.scalar_tensor_tensor(
                    out=oa, in0=po, scalar=1.0, in1=xr_sb[t16],
                    op0=ALU.mult, op1=ALU.add,
                )
                out_acc.append(oa)
            for t16 in range(16):
                po = psB.tile([128, DOUT], F32, name="ps_out", tag="psB")
                for ci, i in enumerate(phaseB):
                    nc.tensor.matmul(
                        out=po,
                        lhsT=cat_tiles[i][:, 128 * t16 : 128 * (t16 + 1)],
                        rhs=outw_sb[i],
                        start=(ci == 0), stop=(ci == len(phaseB) - 1),
                    )
                ro = outpool.tile([128, DOUT], F32, name="ro", tag="ro")
                nc.vector.scalar_tensor_tensor(
                    out=ro, in0=po, scalar=1.0, in1=out_acc[t16],
                    op0=ALU.mult, op1=ALU.add,
                )
                nc.gpsimd.dma_start(out=out[128 * t16 : 128 * (t16 + 1), :], in_=ro)

    if split_waits:
        _split_multi_waits(nc)
    return nc


def _split_multi_waits(nc):
    """This toolchain's walrus encodes at most one sync-wait per instruction;
    hoist extra waits into standalone EventSemaphore instructions on the same
    engine immediately before the original instruction."""
    for bb in nc.main_func.blocks:
        insts = list(bb.instructions)
        if not any(
            ins.sync_info is not None and len(ins.sync_info.on_wait) > 1
            for ins in insts
        ):
            continue
        new = []
        for ins in insts:
            si = ins.sync_info
            if si is not None and len(si.on_wait) > 1:
                waits = list(si.on_wait)
                for k, w in enumerate(waits[:-1]):
                    es = mybir.InstEventSemaphore(name=f"{ins.name}-w{k}", ins=[], outs=[])
                    es.engine = ins.engine
                    es.sync_info = _bass_rust.SyncInfo(on_wait=[w], on_update=[])
                    new.append(es)
                ins.sync_info = _bass_rust.SyncInfo(
                    on_wait=[waits[-1]], on_update=list(si.on_update)
                )
            new.append(ins)
        bb.instructions = new


def _prep_core_inputs(x, norm_gamma, norm_beta, qkv_w, qkv_b, out_w, out_b, core):
    bf16 = ml_dtypes.bfloat16
    b, s = core // 2, core % 2
    heads = [4 * s + i for i in range(H_LOC)]

    # xT: own batch first, remaining batches after (stats are order-invariant)
    other = [bb for bb in range(B) if bb != b]
    xt_cols = [x[b].T] + [x[bb].T for bb in other]
    xT = np.ascontiguousarray(np.concatenate(xt_cols, axis=1)).astype(bf16)

    xres = np.ascontiguousarray(x[b][:, DOUT * s : DOUT * (s + 1)]).astype(np.float32)

    gamma = np.ascontiguousarray(norm_gamma.reshape(DC, 128).T).astype(np.float32)
    beta = np.ascontiguousarray(norm_beta.reshape(DC, 128).T).astype(np.float32)

    wq, wk, wv, wpre = (qkv_w[i * D : (i + 1) * D] for i in range(4))
    bq, bk, bv, bpre = (qkv_b[i * D : (i + 1) * D] for i in range(4))

    qkvwT = np.zeros((D, QKVW_COLS), np.float32)
    qkvb_arr = np.zeros((128, 4), np.float32)
    for m in range(2):
        for hh in range(2):
            hglob = heads[2 * m + hh]
            rows = slice(HD * hglob, HD * (hglob + 1))
            o0 = 64 * hh
            qkvwT[:, QCOL + 128 * m + o0 : QCOL + 128 * m + o0 + HD] = wq[rows].T
            qkvwT[:, KCOL + 128 * m + o0 : KCOL + 128 * m + o0 + HD] = wk[rows].T
            qkvb_arr[o0 : o0 + HD, m] = bq[rows]
            qkvb_arr[o0 : o0 + HD, 2 + m] = bk[rows]
    qkvwT[:, VCOL : VCOL + H_LOC * HD] = wv[HD * heads[0] : HD * (heads[-1] + 1)].T
    qkvwT[:, PCOL : PCOL + D] = wpre.T
    qkvwT = qkvwT.astype(bf16)

    preb = np.ascontiguousarray(bpre.reshape(DC, 128).T).astype(np.float32)
    vb = np.zeros((128, 2), np.float32)
    for m in range(2):
        for hh in range(2):
            hglob = heads[2 * m + hh]
            vb[64 * hh : 64 * hh + HD, m] = bv[HD * hglob : HD * (hglob + 1)]

    # out projection: out_w [D, 768]; rows 0:384 gelu(pre) dims, 384:768 o dims
    owT = out_w.T[:, DOUT * s : DOUT * (s + 1)].astype(np.float32)  # [768, DOUT]
    outwT = np.zeros((7, 128, DOUT), np.float32)
    for m in range(DC):
        outwT[m] = owT[128 * m : 128 * (m + 1)]
    for p2 in range(4):  # o chunks, global head order, heads at row offsets 0/64
        outwT[DC + p2, 0:HD, :] = owT[D + HD * 2 * p2 : D + HD * (2 * p2 + 1)]
        outwT[DC + p2, 64 : 64 + HD, :] = owT[D + HD * (2 * p2 + 1) : D + HD * (2 * p2 + 2)]
    outwT = outwT.astype(bf16)
    outb_arr = out_b[DOUT * s : DOUT * (s + 1)].reshape(1, DOUT).astype(bf16)

    # causal masks for diagonal k-tile pairs: mask_dd[kl, ql] = ql >= kl + 128*dd
    ql = np.arange(QT)[None, :]
    kl = np.arange(128)[:, None]
    m4 = [(ql >= kl + 128 * dd).astype(np.float32) for dd in range(4)]
    masks = np.zeros((2, 128, 1024), np.float32)
    masks[0, :, 0:QT] = m4[0]
    masks[0, :, QT:1024] = m4[1]
    masks[1, :, 0:QT] = m4[2]
    masks[1, :, QT:1024] = m4[3]
    masks = masks.astype(bf16)

    ones_arr = np.ones((1, L), bf16)

    return {
        "xT": xT, "xres": xres, "gamma": gamma, "beta": beta,
        "qkvwT": qkvwT, "qkvb": qkvb_arr, "preb": preb, "vbias": vb,
        "outwT": outwT, "outb": outb_arr, "masks": masks, "ones_row": ones_arr,
    }


def _install_ntff_shim():
    """Provide antenv.axon_hooks (absent in this image) so bass_utils'
    trace path can reach the axon NTFF profiler via ctypes."""
    try:
        import sys, types
        import antenv
        if "antenv.axon_hooks" not in sys.modules:
            from trn_agent_boot.trn_boot import _ntff_profile_via_ctypes
            hook = _ntff_profile_via_ctypes("/opt/axon/libaxon_pjrt.so")
            mod = types.ModuleType("antenv.axon_hooks")
            mod._hook = hook
            mod.set_axon_ntff_profile_hook = lambda h: setattr(mod, "_hook", h)
            mod.get_axon_ntff_profile_hook = lambda: mod._hook
            sys.modules["antenv.axon_hooks"] = mod
            antenv.axon_hooks = mod
        import concourse.bass_utils as _bu
        _bu.upload_artifacts = lambda d: "local"
        return True
    except Exception as e:
        print(f"ntff shim unavailable: {e!r}")
        return False


def kernel(x, norm_gamma, norm_beta, qkv_w, qkv_b, out_w, out_b):
    global LAST_EXEC_TIME_NS, LAST_RESULTS
    x = np.asarray(x, np.float32)
    norm_gamma = np.asarray(norm_gamma, np.float32)
    norm_beta = np.asarray(norm_beta, np.float32)
    qkv_w = np.asarray(qkv_w, np.float32)
    qkv_b = np.asarray(qkv_b, np.float32)
    out_w = np.asarray(out_w, np.float32)
    out_b = np.asarray(out_b, np.float32)

    if "nc" not in _PROGRAM_CACHE:
        _PROGRAM_CACHE["nc"] = _build_program()
    nc = _PROGRAM_CACHE["nc"]

    in_maps = [
        _prep_core_inputs(x, norm_gamma, norm_beta, qkv_w, qkv_b, out_w, out_b, c)
        for c in range(N_CORES)
    ]
    trace = os.environ.get("KERNEL_TRACE", "0") == "1"
    if trace:
        trace = _install_ntff_shim()
    res = run_bass_kernel_spmd(
        nc, in_maps, list(range(N_CORES)), trace=trace,
        trace_cores=list(range(N_CORES)) if trace else None,
    )
    LAST_EXEC_TIME_NS = res.exec_time_ns
    LAST_RESULTS = res
    out = np.empty((B, L, D), np.float32)
    for b in range(B):
        out[b, :, 0:DOUT] = res.results[2 * b]["out"]
        out[b, :, DOUT:D] = res.results[2 * b + 1]["out"]
    return out
